# revision 28
# baseline (speedup 1.0000x reference)
"""AttentionLSTM Trainium2 kernel (v1: bf16 matmuls + restructured softmax).

Sharding: data-parallel over batch. B=32 across 8 cores -> B_local=4 per
core; weights replicated; context/att-context shard with batch.

Per-core layout (all "transposed": feature dim on partitions):
  hT      bf16 [128, 16]  col = kt*4 + b   (d = kt*128 + p)
  cT      f32  [128, 16]
  preact.T accumulates in PSUM f32 [128, 64]  col = mt*4 + b (n = mt*128+p)
  xWT     f32  [128, T*64] col = t*64 + mt*4 + b  (x@W + b, precomputed)
  actx    f32  [128, 2048] col = b*512 + at*128 + l  (a on partitions)
  prep    bf16 [128, 2048] tanh(actx + h_att), same layout
  ctx_sb  bf16 [128, 2048] col = b*512 + c          (l on partitions)
  hsT     bf16 [128, T*16] col = kt*1024 + b*256 + t (h history)

All matmul operands are bf16 (1 PE cycle/row vs 4 for fp32); PSUM
accumulation stays fp32.  Softmax runs in the transposed domain:
prj^T [l=128, b] via prep-as-stationary matmuls, exp on [128,4], row
sums via an all-ones matmul (same value on every partition), so no
PE transposes of alpha and no [1,512]-wide ops on the critical path.

sigmoid(x) = 0.5 + 0.5*tanh(x/2) so the whole kernel only needs the
exp_and_others ACT table set (exp + tanh), loaded once.
Softmax skips max-subtraction: |prj| <= sum|w_att| ~ 20, exp is safe.

PE program order per step is staged so the tensor engine always has
ready work while ACT/DVE run the attention chain:
  Wh | U(kt0,1) | prj | U(kt2) | sum | U(kt3) | wctx | V
"""

import numpy as np
from contextlib import ExitStack

import concourse.bass as bass
import concourse.mybir as mybir
import concourse.tile as tile
from concourse.bass_utils import run_bass_kernel_spmd

F32 = mybir.dt.float32
BF16 = mybir.dt.bfloat16
AF = mybir.ActivationFunctionType
ALU = mybir.AluOpType

B_LOC, T, DIN, D, C, A, L = 4, 256, 512, 512, 512, 512, 128
KT = 4          # 512/128 k-tiles
MT = 16         # 2048/128 m-tiles of the gate dim
NCORES = 8


def split_multi_waits(nc, max_waits=1):
    """This walrus build rejects >1 sync-wait per instruction on some
    opcodes. Hoist extra waits into standalone EventSemaphore preludes."""
    ctr = [0]
    n_fixed = 0

    def fix_block(blk):
        nonlocal n_fixed
        new_insts = []
        for inst in blk.instructions:
            si = inst.sync_info
            waits = list(si.on_wait) if si is not None else []
            if len(waits) > max_waits:
                for w in waits[:-max_waits]:
                    ctr[0] += 1
                    new_insts.append(mybir.InstEventSemaphore(
                        name=f"I-waitsplit-{ctr[0]}",
                        engine=inst.engine, ins=[], outs=[],
                        sync_info=mybir.SyncInfo(on_wait=[w], on_update=[]),
                    ))
                si.on_wait = waits[-max_waits:]
                n_fixed += 1
            new_insts.append(inst)
        blk.instructions[:] = new_insts

    for f in nc.m.functions:
        for blk in f.blocks:
            fix_block(blk)
    return n_fixed


def build_nc(repeat=1):
    nc = bass.Bass()
    x_d = nc.dram_tensor("x", [B_LOC, T, DIN], F32, kind="ExternalInput")
    ctx_d = nc.dram_tensor("context", [B_LOC, L, C], BF16, kind="ExternalInput")
    W_d = nc.dram_tensor("W", [DIN, 4 * D], BF16, kind="ExternalInput")
    V_d = nc.dram_tensor("V", [C, 4 * D], BF16, kind="ExternalInput")
    U_d = nc.dram_tensor("U", [D, 4 * D], BF16, kind="ExternalInput")
    b_d = nc.dram_tensor("b", [4 * D], F32, kind="ExternalInput")
    Wh_d = nc.dram_tensor("W_h_att", [D, A], BF16, kind="ExternalInput")
    Wc_d = nc.dram_tensor("W_ctx_att", [C, A], BF16, kind="ExternalInput")
    ba_d = nc.dram_tensor("b_att", [A], F32, kind="ExternalInput")
    wp_d = nc.dram_tensor("w_att_prj", [A, 1], F32, kind="ExternalInput")
    id_d = nc.dram_tensor("ident", [128, 128], F32, kind="ExternalInput")
    hs_d = nc.dram_tensor("hs", [B_LOC, T, D], F32, kind="ExternalOutput")

    with ExitStack() as ctx:
        tc = ctx.enter_context(tile.TileContext(nc))
        P = ctx.enter_context(tc.tile_pool(name="persist", bufs=1))
        psumP = ctx.enter_context(tc.tile_pool(name="psumP", bufs=1, space="PSUM"))

        # ---------------- persistent tiles ----------------
        xWT = P.tile([128, T * 64], BF16)       # 32KB/part (PE-folded into PSUM)
        hsT = P.tile([128, T * 16], BF16)       # all h_t, col = kt*1024+b*256+t
        idN = P.tile([128, 128], F32)
        idNb = P.tile([128, 128], BF16)
        onesb = P.tile([128, 128], BF16)
        hT = P.tile([128, 16], BF16)            # stores 2*h (see doubled-state note)
        cT = P.tile([128, 16], F32)             # stores 2*c
        hatt = P.tile([128, 16], BF16)
        expT = P.tile([128, 4], BF16)
        r128 = P.tile([128, 4], F32)
        wcnT = P.tile([128, 16], BF16)
        parg = P.tile([128, 2048], BF16)        # actx + h_att (pre-tanh)
        prep = P.tile([128, 2048], BF16)
        dmy = P.tile([128, 4], BF16)            # constant rhs for PE warm-up MMs
        tg = P.tile([128, 64], F32)             # tanh'd gates
        pq = P.tile([128, 32], F32)             # p | q
        tcs = P.tile([128, 16], F32)            # tanh(c)
        h2 = P.tile([128, 16], F32)
        bT = P.tile([128, 16], F32)
        batt = P.tile([128, 4], F32)
        w_sb = P.tile([128, 4], BF16)
        ctx_sb = P.tile([128, 2048], BF16)
        actx = P.tile([128, 2048], BF16)

        pp_pre = psumP.tile([128, 64], F32)
        # small attention tiles share one PSUM bank (bank-granular alloc)
        pp_att = psumP.tile([128, 64], F32)
        pp_hatt = pp_att[:, 0:16]
        pp_prjT = pp_att[:, 16:20]
        pp_sum = pp_att[:, 20:24]
        pp_wc = pp_att[:, 24:40]
        pp_trb = psumP.tile([128, 128], BF16)
        pp_dmy = psumP.tile([128, 4], F32)

        nc.vector.memset(hT[:, :], 0.0)
        nc.vector.memset(cT[:, :], 0.0)
        nc.vector.memset(onesb[:, :], 1.0)
        nc.vector.memset(dmy[:, :], 0.125)

        # natural-layout context load (l on partitions, contiguous rows)
        for b_ in range(B_LOC):
            nc.gpsimd.dma_start(ctx_sb[:, b_ * 512:(b_ + 1) * 512],
                                ctx_d[b_, :, :])

        # ---------------- pre-pass (freed afterwards) ----------------
        with tc.tile_pool(name="pre", bufs=1) as PRE, \
             tc.tile_pool(name="psum_pre", bufs=1, space="PSUM") as psumX:
            xT = PRE.tile([128, 4096], BF16)    # col = kt*1024 + b*256 + t
            x_nat = PRE.tile([128, 4096], F32)  # col = (b*2+th)*512 + d
            W_sb = PRE.tile([128, 8192], BF16)  # col = kt*2048 + m
            Wc_sb = PRE.tile([128, 2048], BF16)  # col = ct*512 + a
            ctxT = PRE.tile([128, 2048], BF16)  # col = b*512 + ct*128 + l

            nc.gpsimd.dma_start(idN[:, :], id_d[:, :])
            nc.vector.tensor_copy(idNb[:, :], idN[:, :])
            for b_ in range(B_LOC):
                for th in range(2):
                    nc.gpsimd.dma_start(
                        x_nat[:, (b_ * 2 + th) * 512:(b_ * 2 + th + 1) * 512],
                        x_d[b_, th * 128:(th + 1) * 128, :])
            # on-chip transpose of x: [t, d] blocks -> [d, t] (f32 -> bf16)
            for b_ in range(B_LOC):
                for th in range(2):
                    for kt in range(KT):
                        pt = psumX.tile([128, 128], F32, tag="pa")
                        nc.tensor.transpose(
                            pt[:, :],
                            x_nat[:, (b_ * 2 + th) * 512 + kt * 128:
                                  (b_ * 2 + th) * 512 + (kt + 1) * 128],
                            idN[:, :])
                        nc.vector.tensor_copy(
                            xT[:, kt * 1024 + b_ * 256 + th * 128:
                               kt * 1024 + b_ * 256 + (th + 1) * 128],
                            pt[:, :])
            for kt in range(KT):
                nc.gpsimd.dma_start(W_sb[:, kt * 2048:(kt + 1) * 2048],
                                    W_d[kt * 128:(kt + 1) * 128, :])
                nc.gpsimd.dma_start(Wc_sb[:, kt * 512:(kt + 1) * 512],
                                    Wc_d[kt * 128:(kt + 1) * 128, :])
            # transposed loads of small vectors via PE (row-major DMA + T)
            bt_nat = PRE.tile([16, 128], F32)
            nc.gpsimd.dma_start(bt_nat[:, :], b_d[:].rearrange("(m p) -> m p", p=128))
            pt = psumX.tile([128, 16], F32, tag="pa")
            nc.tensor.transpose(pt[:, :], bt_nat[:, :], idN[0:16, 0:16])
            nc.vector.tensor_copy(bT[:, :], pt[:, :])
            ba_nat = PRE.tile([4, 128], F32)
            nc.gpsimd.dma_start(ba_nat[:, :], ba_d[:].rearrange("(m p) -> m p", p=128))
            pt = psumX.tile([128, 16], F32, tag="pa")
            nc.tensor.transpose(pt[:, 0:4], ba_nat[:, :], idN[0:4, 0:4])
            nc.vector.tensor_copy(batt[:, :], pt[:, 0:4])
            wp_nat = PRE.tile([4, 128], F32)
            nc.gpsimd.dma_start(wp_nat[:, :],
                                wp_d[:, :].rearrange("(m p) one -> m (p one)", p=128))
            pt = psumX.tile([128, 16], F32, tag="pa")
            nc.tensor.transpose(pt[:, 0:4], wp_nat[:, :], idN[0:4, 0:4])
            nc.vector.tensor_copy(w_sb[:, :], pt[:, 0:4])
            # context transposed (c on partitions) via PE from ctx_sb
            for b_ in range(B_LOC):
                for ct in range(KT):
                    pcb = psumX.tile([128, 128], BF16, tag="pa")
                    nc.tensor.transpose(
                        pcb[:, :],
                        ctx_sb[:, b_ * 512 + ct * 128: b_ * 512 + (ct + 1) * 128],
                        idNb[:, :])
                    nc.vector.tensor_copy(
                        ctxT[:, b_ * 512 + ct * 128: b_ * 512 + (ct + 1) * 128],
                        pcb[:, :])

            # xW^T : per (mtile, b) accumulate over ktiles, N=256 (t)
            xWT3 = xWT[:, :].rearrange("p (t m) -> p t m", m=64)
            for mt in range(MT):
                for b_ in range(B_LOC):
                    px = psumX.tile([128, 256], F32, tag="px")
                    for kt in range(KT):
                        nc.tensor.matmul(
                            px[:, :],
                            lhsT=W_sb[:, kt * 2048 + mt * 128: kt * 2048 + (mt + 1) * 128],
                            rhs=xT[:, kt * 1024 + b_ * 256: kt * 1024 + (b_ + 1) * 256],
                            start=(kt == 0), stop=(kt == KT - 1))
                    # fold LSTM bias b while evacuating PSUM
                    nc.scalar.activation(
                        xWT3[:, :, mt * 4 + b_: mt * 4 + b_ + 1].squeeze(),
                        px[:, :], AF.Identity, bias=bT[:, mt:mt + 1])

            # att_ctx^T = Wctx^T @ ctx^T (+ b_att)
            for b_ in range(B_LOC):
                for at in range(KT):
                    pa = psumX.tile([128, 128], F32, tag="pa")
                    for ct in range(KT):
                        nc.tensor.matmul(
                            pa[:, :],
                            lhsT=Wc_sb[:, ct * 512 + at * 128: ct * 512 + (at + 1) * 128],
                            rhs=ctxT[:, b_ * 512 + ct * 128: b_ * 512 + (ct + 1) * 128],
                            start=(ct == 0), stop=(ct == KT - 1))
                    nc.scalar.activation(
                        actx[:, b_ * 512 + at * 128: b_ * 512 + (at + 1) * 128],
                        pa[:, :], AF.Identity, bias=batt[:, at:at + 1])

        # weights for the scan (allocated after pre-pass frees its space)
        WTS = ctx.enter_context(tc.tile_pool(name="wts", bufs=1))
        U_sb = WTS.tile([128, 8192], BF16)
        V_sb = WTS.tile([128, 8192], BF16)
        Wh_sb = WTS.tile([128, 2048], BF16)
        for kt in range(KT):
            nc.gpsimd.dma_start(U_sb[:, kt * 2048:(kt + 1) * 2048],
                                U_d[kt * 128:(kt + 1) * 128, :])
            nc.gpsimd.dma_start(V_sb[:, kt * 2048:(kt + 1) * 2048],
                                V_d[kt * 128:(kt + 1) * 128, :])
            nc.gpsimd.dma_start(Wh_sb[:, kt * 512:(kt + 1) * 512],
                                Wh_d[kt * 128:(kt + 1) * 128, :])

        # ---------------- the scan ----------------
        import os as _os
        UNROLL = int(_os.environ.get("KERNEL_UNROLL", "8"))
        DUMMY = int(_os.environ.get("KERNEL_DUMMY", "24"))
        TSCAN = int(_os.environ.get("KERNEL_TSCAN", str(T)))

        def u_chunk(kts):
            """U^T h matmuls for the given kt values (16 each)."""
            for kt in kts:
                rhs_h = hT[:, kt * 4:(kt + 1) * 4]
                for mt in range(MT):
                    nc.tensor.matmul(
                        pp_pre[:, mt * 4:(mt + 1) * 4],
                        lhsT=U_sb[:, kt * 2048 + mt * 128: kt * 2048 + (mt + 1) * 128],
                        rhs=rhs_h, start=(kt == 0 and mt == 0), stop=False,
                        skip_group_check=True)

        # broadcast views for the h_att bias add: actx[p, (b at l)] + hatt[p, (at b)]
        actx_v = actx[:, :].rearrange("p (b at l) -> p b at l", b=4, at=4, l=128)
        parg_v = parg[:, :].rearrange("p (b at l) -> p b at l", b=4, at=4, l=128)
        hatt_v = hatt[:, :].rearrange("p (at b) -> p b at", at=4, b=4)
        # wcnT evac folds the softmax normalization: per-column scale r[b]
        wcn_v = wcnT[:, :].rearrange("p (ct b) -> p ct b", ct=4, b=4)
        ppwc_v = pp_wc.rearrange("p (ct b) -> p ct b", ct=4, b=4)
        r_v = r128[:, None, :].broadcast_to([128, 4, 4])

        def step_body(t, xoff=None, hoff=None):
            # --- PE: h_att^T = Wh^T h (16) ---
            for kt in range(KT):
                rhs_h = hT[:, kt * 4:(kt + 1) * 4]
                for ma in range(4):
                    nc.tensor.matmul(
                        pp_hatt[:, ma * 4:(ma + 1) * 4],
                        lhsT=Wh_sb[:, kt * 512 + ma * 128: kt * 512 + (ma + 1) * 128],
                        rhs=rhs_h, start=(kt == 0 and ma == 0),
                        stop=(kt == KT - 1 and ma == 3), skip_group_check=True)
            nc.vector.tensor_copy(hatt[:, :], pp_hatt)

            # --- DVE: parg = actx + h_att (broadcast add, all-bf16 for 2x DVE)
            # --- ACT: prep = tanh(parg) (2 wide ops) ---
            for half in range(2):
                bs = slice(2 * half, 2 * half + 2)
                nc.vector.tensor_tensor(
                    parg_v[:, bs], actx_v[:, bs],
                    hatt_v[:, bs, :, None].broadcast_to([128, 2, 4, 128]),
                    ALU.add)
                nc.scalar.activation(prep[:, 1024 * half:1024 * (half + 1)],
                                     parg[:, 1024 * half:1024 * (half + 1)],
                                     AF.Tanh)

            # --- PE: U chunk A fills the prep window ---
            u_chunk((0, 1))

            # --- PE: prj^T[l, b] = sum_a prep[a, l] w[a] (16, N=1),
            # split so b0/b1 fire after the first tanh half ---
            def prj(b_):
                for at in range(KT):
                    nc.tensor.matmul(
                        pp_prjT[:, b_:b_ + 1],
                        lhsT=prep[:, b_ * 512 + at * 128: b_ * 512 + (at + 1) * 128],
                        rhs=w_sb[:, at:at + 1],
                        start=(at == 0), stop=(at == KT - 1),
                        skip_group_check=True)
            prj(0); prj(1)
            u_chunk((2,))
            prj(2); prj(3)
            u_chunk((3,))

            # --- ACT: exp (no max-subtraction; inputs bounded) ---
            nc.scalar.activation(expT[:, :], pp_prjT[:, :], AF.Exp)
            # --- PE: row sums on every partition via all-ones stationary ---
            nc.tensor.matmul(pp_sum[:, :], lhsT=onesb[:, :], rhs=expT[:, :],
                             start=True, stop=True, skip_group_check=True)
            # --- DVE: r = 1/sums (runs while wctx matmuls stream) ---
            nc.vector.reciprocal(r128[:, :], pp_sum[:, :])

            # --- PE: unnormalized wctx^T[c, b] from expT (ct-major), with the
            # softmax 1/sum folded into per-ct PSUM evacuations so V(kt) can
            # start as soon as its wcnT chunk lands ---
            for ct in range(KT):
                for b_ in range(B_LOC):
                    nc.tensor.matmul(
                        pp_wc[:, ct * 4 + b_: ct * 4 + b_ + 1],
                        lhsT=ctx_sb[:, b_ * 512 + ct * 128: b_ * 512 + (ct + 1) * 128],
                        rhs=expT[:, b_:b_ + 1],
                        start=True, stop=True, skip_group_check=True)
            for ct in range(KT):
                nc.vector.tensor_tensor(wcn_v[:, ct], ppwc_v[:, ct],
                                        r_v[:, ct], ALU.mult)

            # --- PE: preact^T += V^T wctx (64) ---
            for kt in range(KT):
                rhs_w = wcnT[:, kt * 4:(kt + 1) * 4]
                for mt in range(MT):
                    nc.tensor.matmul(
                        pp_pre[:, mt * 4:(mt + 1) * 4],
                        lhsT=V_sb[:, kt * 2048 + mt * 128: kt * 2048 + (mt + 1) * 128],
                        rhs=rhs_w, start=False, stop=False,
                        skip_group_check=True)

            # --- gates. xWT i/f/o region is prescaled (0.5x + 0.5b), so the
            # whole gate stream needs one stt + one add + ONE tanh.
            # xWT folded into PSUM on the PE (identity stationary); ACT then
            # reads the finished preact straight from PSUM.
            xo = t * 64 if xoff is None else xoff
            nc.tensor.matmul(pp_pre[:, :], lhsT=idNb[:, :],
                             rhs=xWT[:, bass.ds(xo, 64)],
                             start=False, stop=True, skip_group_check=True)
            # PE warm-up filler: keeps the clock up through the gates window
            for d_ in range(DUMMY):
                nc.tensor.matmul(pp_dmy[:, :],
                                 lhsT=U_sb[:, (d_ % 64) * 128:(d_ % 64) * 128 + 128],
                                 rhs=dmy[:, :], start=True, stop=True,
                                 skip_group_check=True)
            nc.scalar.activation(tg[:, 0:48], pp_pre[:, 0:48], AF.Tanh, scale=0.5)
            nc.scalar.activation(tg[:, 48:64], pp_pre[:, 48:64], AF.Tanh)
            # Doubled state: cT holds 2c, hT holds 2h (U and W_h_att are
            # pre-halved host-side; the epilogue halves the history).
            #   2c' = 0.5*(tf+1)*(2c) + (ti+1)*cand
            #   2h' = (to+1)*tanh(0.5*(2c'))
            nc.vector.scalar_tensor_tensor(pq[:, 0:16], tg[:, 16:32], 1.0,
                                           cT[:, :], ALU.add, ALU.mult)
            nc.vector.scalar_tensor_tensor(pq[:, 16:32], tg[:, 0:16], 1.0,
                                           tg[:, 48:64], ALU.add, ALU.mult)
            nc.vector.scalar_tensor_tensor(cT[:, :], pq[:, 0:16], 0.5,
                                           pq[:, 16:32], ALU.mult, ALU.add)
            nc.scalar.activation(tcs[:, :], cT[:, :], AF.Tanh, scale=0.5)
            nc.vector.scalar_tensor_tensor(hT[:, :], tg[:, 32:48], 1.0,
                                           tcs[:, :], ALU.add, ALU.mult)

            # --- store h_t into the SBUF history buffer (Pool engine) ---
            hsT4 = hsT[:, :].rearrange("p (k b t) -> p k b t", b=4, t=T)
            nc.gpsimd.tensor_copy(hsT4[:, :, :, bass.ds(t if hoff is None else hoff, 1)].squeeze(), hT[:, :])

        with tc.For_i(0, repeat, 1) as _r, \
             tc.For_i(0, TSCAN // UNROLL, 1) as tb:
            base_x = nc.tensor.snap(tb * (UNROLL * 64))
            base_h = nc.gpsimd.snap(tb * UNROLL)
            for u in range(UNROLL):
                step_body(tb * UNROLL + u,
                          xoff=base_x + u * 64, hoff=base_h + u)

        # ---------------- epilogue: transpose h history, store ----------------
        STG = ctx.enter_context(tc.tile_pool(name="stage", bufs=2))
        for b_ in range(B_LOC):
            for th in range(2):
                st = STG.tile([128, 512], F32, tag="st")
                for kt in range(KT):
                    nc.tensor.transpose(
                        pp_trb[:, :],
                        hsT[:, kt * 1024 + b_ * 256 + th * 128:
                            kt * 1024 + b_ * 256 + (th + 1) * 128],
                        idNb[:, :])
                    # halve: hsT holds 2h (doubled-state trick)
                    nc.vector.tensor_scalar_mul(st[:, kt * 128:(kt + 1) * 128],
                                                pp_trb[:, :], 0.5)
                nc.sync.dma_start(hs_d[b_, th * 128:(th + 1) * 128, :], st[:, :])

    split_multi_waits(nc)
    return nc


_NC_CACHE = {}


def _get_nc(repeat=1):
    if repeat not in _NC_CACHE:
        _NC_CACHE[repeat] = build_nc(repeat)
    return _NC_CACHE[repeat]


def kernel(x, context, W, V, U, b, W_h_att, W_ctx_att, b_att, w_att_prj,
           bench_repeat=1, **run_kwargs):
    import ml_dtypes
    BF = ml_dtypes.bfloat16
    nc = _get_nc(bench_repeat)
    f32 = lambda a: np.ascontiguousarray(np.asarray(a), dtype=np.float32)
    bf16 = lambda a: np.ascontiguousarray(np.asarray(a, dtype=np.float32).astype(BF))
    x, context = f32(x), np.asarray(context, dtype=np.float32)
    # U and W_h_att pre-halved: the device state tiles hold 2h/2c
    shared = dict(W=bf16(W), V=bf16(V), U=bf16(np.asarray(U) * 0.5), b=f32(b),
                  W_h_att=bf16(np.asarray(W_h_att) * 0.5), W_ctx_att=bf16(W_ctx_att),
                  b_att=f32(b_att), w_att_prj=f32(w_att_prj),
                  ident=np.eye(128, dtype=np.float32))
    in_maps = []
    for c in range(NCORES):
        m = dict(shared)
        m["x"] = np.ascontiguousarray(x[c * B_LOC:(c + 1) * B_LOC])
        m["context"] = bf16(context[c * B_LOC:(c + 1) * B_LOC])
        in_maps.append(m)
    res = run_bass_kernel_spmd(nc, in_maps, core_ids=list(range(NCORES)),
                               **run_kwargs)
    out = np.concatenate([r["hs"] for r in res.results], axis=0)
    kernel.last_result = res
    return out


if __name__ == "__main__":
    rng = np.random.default_rng(0)
    ins = {
        "x": rng.standard_normal((32, T, DIN), dtype=np.float32),
        "context": rng.standard_normal((32, L, C), dtype=np.float32),
        "W": (rng.standard_normal((DIN, 4 * D), dtype=np.float32) * 0.05),
        "V": (rng.standard_normal((C, 4 * D), dtype=np.float32) * 0.05),
        "U": (rng.standard_normal((D, 4 * D), dtype=np.float32) * 0.05),
        "b": np.zeros(4 * D, np.float32),
        "W_h_att": (rng.standard_normal((D, A), dtype=np.float32) * 0.05),
        "W_ctx_att": (rng.standard_normal((C, A), dtype=np.float32) * 0.05),
        "b_att": np.zeros(A, np.float32),
        "w_att_prj": (rng.standard_normal((A, 1), dtype=np.float32) * 0.05),
    }
    out = kernel(**ins)
    print("out", out.shape, out.dtype, float(np.abs(out).max()))


# revision 29
# speedup vs baseline: 2.8799x; 2.8799x over previous
"""AttentionLSTM Trainium2 kernel (v1: bf16 matmuls + restructured softmax).

Sharding: data-parallel over batch. B=32 across 8 cores -> B_local=4 per
core; weights replicated; context/att-context shard with batch.

Per-core layout (all "transposed": feature dim on partitions):
  hT      bf16 [128, 16]  col = kt*4 + b   (d = kt*128 + p)
  cT      f32  [128, 16]
  preact.T accumulates in PSUM f32 [128, 64]  col = mt*4 + b (n = mt*128+p)
  xWT     f32  [128, T*64] col = t*64 + mt*4 + b  (x@W + b, precomputed)
  actx    f32  [128, 2048] col = b*512 + at*128 + l  (a on partitions)
  prep    bf16 [128, 2048] tanh(actx + h_att), same layout
  ctx_sb  bf16 [128, 2048] col = b*512 + c          (l on partitions)
  hsT     bf16 [128, T*16] col = kt*1024 + b*256 + t (h history)

All matmul operands are bf16 (1 PE cycle/row vs 4 for fp32); PSUM
accumulation stays fp32.  Softmax runs in the transposed domain:
prj^T [l=128, b] via prep-as-stationary matmuls, exp on [128,4], row
sums via an all-ones matmul (same value on every partition), so no
PE transposes of alpha and no [1,512]-wide ops on the critical path.

sigmoid(x) = 0.5 + 0.5*tanh(x/2) so the whole kernel only needs the
exp_and_others ACT table set (exp + tanh), loaded once.
Softmax skips max-subtraction: |prj| <= sum|w_att| ~ 20, exp is safe.

PE program order per step is staged so the tensor engine always has
ready work while ACT/DVE run the attention chain:
  Wh | U(kt0,1) | prj | U(kt2) | sum | U(kt3) | wctx | V
"""

import numpy as np
from contextlib import ExitStack

import concourse.bass as bass
import concourse.mybir as mybir
import concourse.tile as tile
from concourse.bass_utils import run_bass_kernel_spmd

F32 = mybir.dt.float32
BF16 = mybir.dt.bfloat16
AF = mybir.ActivationFunctionType
ALU = mybir.AluOpType

B_LOC, T, DIN, D, C, A, L = 4, 256, 512, 512, 512, 512, 128
KT = 4          # 512/128 k-tiles
MT = 16         # 2048/128 m-tiles of the gate dim
NCORES = 8


def split_multi_waits(nc, max_waits=1):
    """This walrus build rejects >1 sync-wait per instruction on some
    opcodes. Hoist extra waits into standalone EventSemaphore preludes."""
    ctr = [0]
    n_fixed = 0

    def fix_block(blk):
        nonlocal n_fixed
        new_insts = []
        for inst in blk.instructions:
            si = inst.sync_info
            waits = list(si.on_wait) if si is not None else []
            if len(waits) > max_waits:
                for w in waits[:-max_waits]:
                    ctr[0] += 1
                    new_insts.append(mybir.InstEventSemaphore(
                        name=f"I-waitsplit-{ctr[0]}",
                        engine=inst.engine, ins=[], outs=[],
                        sync_info=mybir.SyncInfo(on_wait=[w], on_update=[]),
                    ))
                si.on_wait = waits[-max_waits:]
                n_fixed += 1
            new_insts.append(inst)
        blk.instructions[:] = new_insts

    for f in nc.m.functions:
        for blk in f.blocks:
            fix_block(blk)
    return n_fixed


def build_nc(repeat=1):
    nc = bass.Bass()
    x_d = nc.dram_tensor("x", [B_LOC, T, DIN], F32, kind="ExternalInput")
    ctx_d = nc.dram_tensor("context", [B_LOC, L, C], BF16, kind="ExternalInput")
    W_d = nc.dram_tensor("W", [DIN, 4 * D], BF16, kind="ExternalInput")
    V_d = nc.dram_tensor("V", [C, 4 * D], BF16, kind="ExternalInput")
    U_d = nc.dram_tensor("U", [D, 4 * D], BF16, kind="ExternalInput")
    b_d = nc.dram_tensor("b", [4 * D], F32, kind="ExternalInput")
    Wh_d = nc.dram_tensor("W_h_att", [D, A], BF16, kind="ExternalInput")
    Wc_d = nc.dram_tensor("W_ctx_att", [C, A], BF16, kind="ExternalInput")
    ba_d = nc.dram_tensor("b_att", [A], F32, kind="ExternalInput")
    wp_d = nc.dram_tensor("w_att_prj", [A, 1], F32, kind="ExternalInput")
    id_d = nc.dram_tensor("ident", [128, 128], F32, kind="ExternalInput")
    hs_d = nc.dram_tensor("hs", [B_LOC, T, D], F32, kind="ExternalOutput")

    with ExitStack() as ctx:
        tc = ctx.enter_context(tile.TileContext(nc))
        P = ctx.enter_context(tc.tile_pool(name="persist", bufs=1))
        psumP = ctx.enter_context(tc.tile_pool(name="psumP", bufs=1, space="PSUM"))

        # ---------------- persistent tiles ----------------
        xWT = P.tile([128, T * 64], BF16)       # 32KB/part (PE-folded into PSUM)
        hsT = P.tile([128, T * 16], BF16)       # all h_t, col = kt*1024+b*256+t
        idN = P.tile([128, 128], F32)
        idNb = P.tile([128, 128], BF16)
        onesb = P.tile([128, 128], BF16)
        hT = P.tile([128, 16], BF16)            # stores 2*h (see doubled-state note)
        cT = P.tile([128, 16], F32)             # stores 2*c
        hatt = P.tile([128, 16], BF16)
        expT = P.tile([128, 4], BF16)
        r128 = P.tile([128, 4], F32)
        wcnT = P.tile([128, 16], BF16)
        parg = P.tile([128, 2048], BF16)        # actx + h_att (pre-tanh)
        prep = P.tile([128, 2048], BF16)
        garg = P.tile([128, 64], F32)
        dmy = P.tile([128, 4], BF16)            # constant rhs for PE warm-up MMs
        tg = P.tile([128, 64], F32)             # tanh'd gates
        pq = P.tile([128, 32], F32)             # p | q
        tcs = P.tile([128, 16], F32)            # tanh(c)
        h2 = P.tile([128, 16], F32)
        bT = P.tile([128, 16], F32)
        batt = P.tile([128, 4], F32)
        w_sb = P.tile([128, 4], BF16)
        ctx_sb = P.tile([128, 2048], BF16)
        actx = P.tile([128, 2048], BF16)

        pp_pre = psumP.tile([128, 64], F32)
        # small attention tiles share one PSUM bank (bank-granular alloc)
        pp_att = psumP.tile([128, 64], F32)
        pp_hatt = pp_att[:, 0:16]
        pp_prjT = pp_att[:, 16:20]
        pp_sum = pp_att[:, 20:24]
        pp_wc = pp_att[:, 24:40]
        pp_trb = psumP.tile([128, 128], BF16)
        pp_dmy = psumP.tile([128, 4], F32)

        nc.vector.memset(hT[:, :], 0.0)
        nc.vector.memset(cT[:, :], 0.0)
        nc.vector.memset(onesb[:, :], 1.0)
        nc.vector.memset(dmy[:, :], 0.125)

        # natural-layout context load (l on partitions, contiguous rows)
        for b_ in range(B_LOC):
            nc.gpsimd.dma_start(ctx_sb[:, b_ * 512:(b_ + 1) * 512],
                                ctx_d[b_, :, :])

        # ---------------- pre-pass (freed afterwards) ----------------
        with tc.tile_pool(name="pre", bufs=1) as PRE, \
             tc.tile_pool(name="psum_pre", bufs=1, space="PSUM") as psumX:
            xT = PRE.tile([128, 4096], BF16)    # col = kt*1024 + b*256 + t
            x_nat = PRE.tile([128, 4096], F32)  # col = (b*2+th)*512 + d
            W_sb = PRE.tile([128, 8192], BF16)  # col = kt*2048 + m
            Wc_sb = PRE.tile([128, 2048], BF16)  # col = ct*512 + a
            ctxT = PRE.tile([128, 2048], BF16)  # col = b*512 + ct*128 + l

            nc.gpsimd.dma_start(idN[:, :], id_d[:, :])
            nc.vector.tensor_copy(idNb[:, :], idN[:, :])
            for b_ in range(B_LOC):
                for th in range(2):
                    nc.gpsimd.dma_start(
                        x_nat[:, (b_ * 2 + th) * 512:(b_ * 2 + th + 1) * 512],
                        x_d[b_, th * 128:(th + 1) * 128, :])
            # on-chip transpose of x: [t, d] blocks -> [d, t] (f32 -> bf16)
            for b_ in range(B_LOC):
                for th in range(2):
                    for kt in range(KT):
                        pt = psumX.tile([128, 128], F32, tag="pa")
                        nc.tensor.transpose(
                            pt[:, :],
                            x_nat[:, (b_ * 2 + th) * 512 + kt * 128:
                                  (b_ * 2 + th) * 512 + (kt + 1) * 128],
                            idN[:, :])
                        nc.vector.tensor_copy(
                            xT[:, kt * 1024 + b_ * 256 + th * 128:
                               kt * 1024 + b_ * 256 + (th + 1) * 128],
                            pt[:, :])
            for kt in range(KT):
                nc.gpsimd.dma_start(W_sb[:, kt * 2048:(kt + 1) * 2048],
                                    W_d[kt * 128:(kt + 1) * 128, :])
                nc.gpsimd.dma_start(Wc_sb[:, kt * 512:(kt + 1) * 512],
                                    Wc_d[kt * 128:(kt + 1) * 128, :])
            # transposed loads of small vectors via PE (row-major DMA + T)
            bt_nat = PRE.tile([16, 128], F32)
            nc.gpsimd.dma_start(bt_nat[:, :], b_d[:].rearrange("(m p) -> m p", p=128))
            pt = psumX.tile([128, 16], F32, tag="pa")
            nc.tensor.transpose(pt[:, :], bt_nat[:, :], idN[0:16, 0:16])
            nc.vector.tensor_copy(bT[:, :], pt[:, :])
            ba_nat = PRE.tile([4, 128], F32)
            nc.gpsimd.dma_start(ba_nat[:, :], ba_d[:].rearrange("(m p) -> m p", p=128))
            pt = psumX.tile([128, 16], F32, tag="pa")
            nc.tensor.transpose(pt[:, 0:4], ba_nat[:, :], idN[0:4, 0:4])
            nc.vector.tensor_copy(batt[:, :], pt[:, 0:4])
            wp_nat = PRE.tile([4, 128], F32)
            nc.gpsimd.dma_start(wp_nat[:, :],
                                wp_d[:, :].rearrange("(m p) one -> m (p one)", p=128))
            pt = psumX.tile([128, 16], F32, tag="pa")
            nc.tensor.transpose(pt[:, 0:4], wp_nat[:, :], idN[0:4, 0:4])
            nc.vector.tensor_copy(w_sb[:, :], pt[:, 0:4])
            # context transposed (c on partitions) via PE from ctx_sb
            for b_ in range(B_LOC):
                for ct in range(KT):
                    pcb = psumX.tile([128, 128], BF16, tag="pa")
                    nc.tensor.transpose(
                        pcb[:, :],
                        ctx_sb[:, b_ * 512 + ct * 128: b_ * 512 + (ct + 1) * 128],
                        idNb[:, :])
                    nc.vector.tensor_copy(
                        ctxT[:, b_ * 512 + ct * 128: b_ * 512 + (ct + 1) * 128],
                        pcb[:, :])

            # xW^T : per (mtile, b) accumulate over ktiles, N=256 (t)
            xWT3 = xWT[:, :].rearrange("p (t m) -> p t m", m=64)
            for mt in range(MT):
                for b_ in range(B_LOC):
                    px = psumX.tile([128, 256], F32, tag="px")
                    for kt in range(KT):
                        nc.tensor.matmul(
                            px[:, :],
                            lhsT=W_sb[:, kt * 2048 + mt * 128: kt * 2048 + (mt + 1) * 128],
                            rhs=xT[:, kt * 1024 + b_ * 256: kt * 1024 + (b_ + 1) * 256],
                            start=(kt == 0), stop=(kt == KT - 1))
                    # fold LSTM bias b while evacuating PSUM
                    nc.scalar.activation(
                        xWT3[:, :, mt * 4 + b_: mt * 4 + b_ + 1].squeeze(),
                        px[:, :], AF.Identity, bias=bT[:, mt:mt + 1])

            # att_ctx^T = Wctx^T @ ctx^T (+ b_att)
            for b_ in range(B_LOC):
                for at in range(KT):
                    pa = psumX.tile([128, 128], F32, tag="pa")
                    for ct in range(KT):
                        nc.tensor.matmul(
                            pa[:, :],
                            lhsT=Wc_sb[:, ct * 512 + at * 128: ct * 512 + (at + 1) * 128],
                            rhs=ctxT[:, b_ * 512 + ct * 128: b_ * 512 + (ct + 1) * 128],
                            start=(ct == 0), stop=(ct == KT - 1))
                    nc.scalar.activation(
                        actx[:, b_ * 512 + at * 128: b_ * 512 + (at + 1) * 128],
                        pa[:, :], AF.Identity, bias=batt[:, at:at + 1])

        # weights for the scan (allocated after pre-pass frees its space)
        WTS = ctx.enter_context(tc.tile_pool(name="wts", bufs=1))
        U_sb = WTS.tile([128, 8192], BF16)
        V_sb = WTS.tile([128, 8192], BF16)
        Wh_sb = WTS.tile([128, 2048], BF16)
        for kt in range(KT):
            nc.gpsimd.dma_start(U_sb[:, kt * 2048:(kt + 1) * 2048],
                                U_d[kt * 128:(kt + 1) * 128, :])
            nc.gpsimd.dma_start(V_sb[:, kt * 2048:(kt + 1) * 2048],
                                V_d[kt * 128:(kt + 1) * 128, :])
            nc.gpsimd.dma_start(Wh_sb[:, kt * 512:(kt + 1) * 512],
                                Wh_d[kt * 128:(kt + 1) * 128, :])

        # ---------------- the scan ----------------
        import os as _os
        UNROLL = int(_os.environ.get("KERNEL_UNROLL", "8"))
        DUMMY = int(_os.environ.get("KERNEL_DUMMY", "24"))
        TSCAN = int(_os.environ.get("KERNEL_TSCAN", str(T)))

        def u_chunk(kts):
            """U^T h matmuls for the given kt values (16 each)."""
            for kt in kts:
                rhs_h = hT[:, kt * 4:(kt + 1) * 4]
                for mt in range(MT):
                    nc.tensor.matmul(
                        pp_pre[:, mt * 4:(mt + 1) * 4],
                        lhsT=U_sb[:, kt * 2048 + mt * 128: kt * 2048 + (mt + 1) * 128],
                        rhs=rhs_h, start=(kt == 0 and mt == 0), stop=False,
                        skip_group_check=True)

        # broadcast views for the h_att bias add: actx[p, (b at l)] + hatt[p, (at b)]
        actx_v = actx[:, :].rearrange("p (b at l) -> p b at l", b=4, at=4, l=128)
        parg_v = parg[:, :].rearrange("p (b at l) -> p b at l", b=4, at=4, l=128)
        hatt_v = hatt[:, :].rearrange("p (at b) -> p b at", at=4, b=4)
        # wcnT evac folds the softmax normalization: per-column scale r[b]
        wcn_v = wcnT[:, :].rearrange("p (ct b) -> p ct b", ct=4, b=4)
        ppwc_v = pp_wc.rearrange("p (ct b) -> p ct b", ct=4, b=4)
        r_v = r128[:, None, :].broadcast_to([128, 4, 4])

        def step_body(t, xoff=None, hoff=None):
            # --- PE: h_att^T = Wh^T h (16) ---
            for kt in range(KT):
                rhs_h = hT[:, kt * 4:(kt + 1) * 4]
                for ma in range(4):
                    nc.tensor.matmul(
                        pp_hatt[:, ma * 4:(ma + 1) * 4],
                        lhsT=Wh_sb[:, kt * 512 + ma * 128: kt * 512 + (ma + 1) * 128],
                        rhs=rhs_h, start=(kt == 0 and ma == 0),
                        stop=(kt == KT - 1 and ma == 3), skip_group_check=True)
            nc.vector.tensor_copy(hatt[:, :], pp_hatt)

            # --- DVE: parg = actx + h_att (broadcast add, all-bf16 for 2x DVE)
            # --- ACT: prep = tanh(parg) (2 wide ops) ---
            for half in range(2):
                bs = slice(2 * half, 2 * half + 2)
                nc.vector.tensor_tensor(
                    parg_v[:, bs], actx_v[:, bs],
                    hatt_v[:, bs, :, None].broadcast_to([128, 2, 4, 128]),
                    ALU.add)
                nc.scalar.activation(prep[:, 1024 * half:1024 * (half + 1)],
                                     parg[:, 1024 * half:1024 * (half + 1)],
                                     AF.Tanh)

            # --- PE: U chunk A fills the prep window ---
            u_chunk((0, 1))

            # --- PE: prj^T[l, b] = sum_a prep[a, l] w[a] (16, N=1),
            # split so b0/b1 fire after the first tanh half ---
            def prj(b_):
                for at in range(KT):
                    nc.tensor.matmul(
                        pp_prjT[:, b_:b_ + 1],
                        lhsT=prep[:, b_ * 512 + at * 128: b_ * 512 + (at + 1) * 128],
                        rhs=w_sb[:, at:at + 1],
                        start=(at == 0), stop=(at == KT - 1),
                        skip_group_check=True)
            prj(0); prj(1)
            u_chunk((2,))
            prj(2); prj(3)
            u_chunk((3,))

            # --- ACT: exp (no max-subtraction; inputs bounded) ---
            nc.scalar.activation(expT[:, :], pp_prjT[:, :], AF.Exp)
            # --- PE: row sums on every partition via all-ones stationary ---
            nc.tensor.matmul(pp_sum[:, :], lhsT=onesb[:, :], rhs=expT[:, :],
                             start=True, stop=True, skip_group_check=True)
            # --- DVE: r = 1/sums (runs while wctx matmuls stream) ---
            nc.vector.reciprocal(r128[:, :], pp_sum[:, :])

            # --- PE: unnormalized wctx^T[c, b] from expT (ct-major), with the
            # softmax 1/sum folded into per-ct PSUM evacuations so V(kt) can
            # start as soon as its wcnT chunk lands ---
            for ct in range(KT):
                for b_ in range(B_LOC):
                    nc.tensor.matmul(
                        pp_wc[:, ct * 4 + b_: ct * 4 + b_ + 1],
                        lhsT=ctx_sb[:, b_ * 512 + ct * 128: b_ * 512 + (ct + 1) * 128],
                        rhs=expT[:, b_:b_ + 1],
                        start=True, stop=True, skip_group_check=True)
            for ct in range(KT):
                nc.vector.tensor_tensor(wcn_v[:, ct], ppwc_v[:, ct],
                                        r_v[:, ct], ALU.mult)

            # --- PE: preact^T += V^T wctx (64) ---
            for kt in range(KT):
                rhs_w = wcnT[:, kt * 4:(kt + 1) * 4]
                for mt in range(MT):
                    nc.tensor.matmul(
                        pp_pre[:, mt * 4:(mt + 1) * 4],
                        lhsT=V_sb[:, kt * 2048 + mt * 128: kt * 2048 + (mt + 1) * 128],
                        rhs=rhs_w,
                        start=False, stop=(kt == KT - 1 and mt == MT - 1),
                        skip_group_check=True)

            # --- gates. xWT i/f/o region is prescaled (0.5x + 0.5b), so the
            # whole gate stream needs one stt + one add + ONE tanh.
            # PE warm-up filler: keeps the clock up through the gates window
            for d_ in range(DUMMY):
                nc.tensor.matmul(pp_dmy[:, :],
                                 lhsT=U_sb[:, (d_ % 64) * 128:(d_ % 64) * 128 + 128],
                                 rhs=dmy[:, :], start=True, stop=True,
                                 skip_group_check=True)
            xo = t * 64 if xoff is None else xoff
            nc.vector.tensor_add(garg[:, :], pp_pre[:, :], xWT[:, bass.ds(xo, 64)])
            nc.scalar.activation(tg[:, 0:48], garg[:, 0:48], AF.Tanh, scale=0.5)
            nc.scalar.activation(tg[:, 48:64], garg[:, 48:64], AF.Tanh)
            # Doubled state: cT holds 2c, hT holds 2h (U and W_h_att are
            # pre-halved host-side; the epilogue halves the history).
            #   2c' = 0.5*(tf+1)*(2c) + (ti+1)*cand
            #   2h' = (to+1)*tanh(0.5*(2c'))
            nc.vector.scalar_tensor_tensor(pq[:, 0:16], tg[:, 16:32], 1.0,
                                           cT[:, :], ALU.add, ALU.mult)
            nc.vector.scalar_tensor_tensor(pq[:, 16:32], tg[:, 0:16], 1.0,
                                           tg[:, 48:64], ALU.add, ALU.mult)
            nc.vector.scalar_tensor_tensor(cT[:, :], pq[:, 0:16], 0.5,
                                           pq[:, 16:32], ALU.mult, ALU.add)
            nc.scalar.activation(tcs[:, :], cT[:, :], AF.Tanh, scale=0.5)
            nc.vector.scalar_tensor_tensor(hT[:, :], tg[:, 32:48], 1.0,
                                           tcs[:, :], ALU.add, ALU.mult)

            # --- store h_t into the SBUF history buffer (Pool engine) ---
            hsT4 = hsT[:, :].rearrange("p (k b t) -> p k b t", b=4, t=T)
            nc.gpsimd.tensor_copy(hsT4[:, :, :, bass.ds(t if hoff is None else hoff, 1)].squeeze(), hT[:, :])

        with tc.For_i(0, repeat, 1) as _r, \
             tc.For_i(0, TSCAN // UNROLL, 1) as tb:
            base_x = nc.vector.snap(tb * (UNROLL * 64))
            base_h = nc.gpsimd.snap(tb * UNROLL)
            for u in range(UNROLL):
                step_body(tb * UNROLL + u,
                          xoff=base_x + u * 64, hoff=base_h + u)

        # ---------------- epilogue: transpose h history, store ----------------
        STG = ctx.enter_context(tc.tile_pool(name="stage", bufs=2))
        for b_ in range(B_LOC):
            for th in range(2):
                st = STG.tile([128, 512], F32, tag="st")
                for kt in range(KT):
                    nc.tensor.transpose(
                        pp_trb[:, :],
                        hsT[:, kt * 1024 + b_ * 256 + th * 128:
                            kt * 1024 + b_ * 256 + (th + 1) * 128],
                        idNb[:, :])
                    # halve: hsT holds 2h (doubled-state trick)
                    nc.vector.tensor_scalar_mul(st[:, kt * 128:(kt + 1) * 128],
                                                pp_trb[:, :], 0.5)
                nc.sync.dma_start(hs_d[b_, th * 128:(th + 1) * 128, :], st[:, :])

    split_multi_waits(nc)
    return nc


_NC_CACHE = {}


def _get_nc(repeat=1):
    if repeat not in _NC_CACHE:
        _NC_CACHE[repeat] = build_nc(repeat)
    return _NC_CACHE[repeat]


def kernel(x, context, W, V, U, b, W_h_att, W_ctx_att, b_att, w_att_prj,
           bench_repeat=1, **run_kwargs):
    import ml_dtypes
    BF = ml_dtypes.bfloat16
    nc = _get_nc(bench_repeat)
    f32 = lambda a: np.ascontiguousarray(np.asarray(a), dtype=np.float32)
    bf16 = lambda a: np.ascontiguousarray(np.asarray(a, dtype=np.float32).astype(BF))
    x, context = f32(x), np.asarray(context, dtype=np.float32)
    # U and W_h_att pre-halved: the device state tiles hold 2h/2c
    shared = dict(W=bf16(W), V=bf16(V), U=bf16(np.asarray(U) * 0.5), b=f32(b),
                  W_h_att=bf16(np.asarray(W_h_att) * 0.5), W_ctx_att=bf16(W_ctx_att),
                  b_att=f32(b_att), w_att_prj=f32(w_att_prj),
                  ident=np.eye(128, dtype=np.float32))
    in_maps = []
    for c in range(NCORES):
        m = dict(shared)
        m["x"] = np.ascontiguousarray(x[c * B_LOC:(c + 1) * B_LOC])
        m["context"] = bf16(context[c * B_LOC:(c + 1) * B_LOC])
        in_maps.append(m)
    res = run_bass_kernel_spmd(nc, in_maps, core_ids=list(range(NCORES)),
                               **run_kwargs)
    out = np.concatenate([r["hs"] for r in res.results], axis=0)
    kernel.last_result = res
    return out


if __name__ == "__main__":
    rng = np.random.default_rng(0)
    ins = {
        "x": rng.standard_normal((32, T, DIN), dtype=np.float32),
        "context": rng.standard_normal((32, L, C), dtype=np.float32),
        "W": (rng.standard_normal((DIN, 4 * D), dtype=np.float32) * 0.05),
        "V": (rng.standard_normal((C, 4 * D), dtype=np.float32) * 0.05),
        "U": (rng.standard_normal((D, 4 * D), dtype=np.float32) * 0.05),
        "b": np.zeros(4 * D, np.float32),
        "W_h_att": (rng.standard_normal((D, A), dtype=np.float32) * 0.05),
        "W_ctx_att": (rng.standard_normal((C, A), dtype=np.float32) * 0.05),
        "b_att": np.zeros(A, np.float32),
        "w_att_prj": (rng.standard_normal((A, 1), dtype=np.float32) * 0.05),
    }
    out = kernel(**ins)
    print("out", out.shape, out.dtype, float(np.abs(out).max()))


# revision 30
# speedup vs baseline: 3.6618x; 1.2715x over previous
"""AttentionLSTM Trainium2 kernel (v1: bf16 matmuls + restructured softmax).

Sharding: data-parallel over batch. B=32 across 8 cores -> B_local=4 per
core; weights replicated; context/att-context shard with batch.

Per-core layout (all "transposed": feature dim on partitions):
  hT      bf16 [128, 16]  col = kt*4 + b   (d = kt*128 + p)
  cT      f32  [128, 16]
  preact.T accumulates in PSUM f32 [128, 64]  col = mt*4 + b (n = mt*128+p)
  xWT     f32  [128, T*64] col = t*64 + mt*4 + b  (x@W + b, precomputed)
  actx    f32  [128, 2048] col = b*512 + at*128 + l  (a on partitions)
  prep    bf16 [128, 2048] tanh(actx + h_att), same layout
  ctx_sb  bf16 [128, 2048] col = b*512 + c          (l on partitions)
  hsT     bf16 [128, T*16] col = kt*1024 + b*256 + t (h history)

All matmul operands are bf16 (1 PE cycle/row vs 4 for fp32); PSUM
accumulation stays fp32.  Softmax runs in the transposed domain:
prj^T [l=128, b] via prep-as-stationary matmuls, exp on [128,4], row
sums via an all-ones matmul (same value on every partition), so no
PE transposes of alpha and no [1,512]-wide ops on the critical path.

sigmoid(x) = 0.5 + 0.5*tanh(x/2) so the whole kernel only needs the
exp_and_others ACT table set (exp + tanh), loaded once.
Softmax skips max-subtraction: |prj| <= sum|w_att| ~ 20, exp is safe.

PE program order per step is staged so the tensor engine always has
ready work while ACT/DVE run the attention chain:
  Wh | U(kt0,1) | prj | U(kt2) | sum | U(kt3) | wctx | V
"""

import numpy as np
from contextlib import ExitStack

import concourse.bass as bass
import concourse.mybir as mybir
import concourse.tile as tile
from concourse.bass_utils import run_bass_kernel_spmd

F32 = mybir.dt.float32
BF16 = mybir.dt.bfloat16
AF = mybir.ActivationFunctionType
ALU = mybir.AluOpType

B_LOC, T, DIN, D, C, A, L = 4, 256, 512, 512, 512, 512, 128
KT = 4          # 512/128 k-tiles
MT = 16         # 2048/128 m-tiles of the gate dim
NCORES = 8


def split_multi_waits(nc, max_waits=1):
    """This walrus build rejects >1 sync-wait per instruction on some
    opcodes. Hoist extra waits into standalone EventSemaphore preludes."""
    ctr = [0]
    n_fixed = 0

    def fix_block(blk):
        nonlocal n_fixed
        new_insts = []
        for inst in blk.instructions:
            si = inst.sync_info
            waits = list(si.on_wait) if si is not None else []
            if len(waits) > max_waits:
                for w in waits[:-max_waits]:
                    ctr[0] += 1
                    new_insts.append(mybir.InstEventSemaphore(
                        name=f"I-waitsplit-{ctr[0]}",
                        engine=inst.engine, ins=[], outs=[],
                        sync_info=mybir.SyncInfo(on_wait=[w], on_update=[]),
                    ))
                si.on_wait = waits[-max_waits:]
                n_fixed += 1
            new_insts.append(inst)
        blk.instructions[:] = new_insts

    for f in nc.m.functions:
        for blk in f.blocks:
            fix_block(blk)
    return n_fixed


def build_nc(repeat=1):
    nc = bass.Bass()
    x_d = nc.dram_tensor("x", [B_LOC, T, DIN], F32, kind="ExternalInput")
    ctx_d = nc.dram_tensor("context", [B_LOC, L, C], BF16, kind="ExternalInput")
    W_d = nc.dram_tensor("W", [DIN, 4 * D], BF16, kind="ExternalInput")
    V_d = nc.dram_tensor("V", [C, 4 * D], BF16, kind="ExternalInput")
    U_d = nc.dram_tensor("U", [D, 4 * D], BF16, kind="ExternalInput")
    b_d = nc.dram_tensor("b", [4 * D], F32, kind="ExternalInput")
    Wh_d = nc.dram_tensor("W_h_att", [D, A], BF16, kind="ExternalInput")
    Wc_d = nc.dram_tensor("W_ctx_att", [C, A], BF16, kind="ExternalInput")
    ba_d = nc.dram_tensor("b_att", [A], F32, kind="ExternalInput")
    wp_d = nc.dram_tensor("w_att_prj", [A, 1], F32, kind="ExternalInput")
    id_d = nc.dram_tensor("ident", [128, 128], F32, kind="ExternalInput")
    hs_d = nc.dram_tensor("hs", [B_LOC, T, D], F32, kind="ExternalOutput")

    with ExitStack() as ctx:
        tc = ctx.enter_context(tile.TileContext(nc))
        P = ctx.enter_context(tc.tile_pool(name="persist", bufs=1))
        psumP = ctx.enter_context(tc.tile_pool(name="psumP", bufs=1, space="PSUM"))

        # ---------------- persistent tiles ----------------
        xWT = P.tile([128, T * 64], BF16)       # 32KB/part (PE-folded into PSUM)
        hsT = P.tile([128, T * 16], BF16)       # all h_t, col = kt*1024+b*256+t
        idN = P.tile([128, 128], F32)
        idNb = P.tile([128, 128], BF16)
        onesb = P.tile([128, 128], BF16)
        hT = P.tile([128, 16], BF16)            # stores 2*h (see doubled-state note)
        cT = P.tile([128, 16], F32)             # stores 2*c
        expT = P.tile([128, 4], BF16)
        r128 = P.tile([128, 4], F32)
        wcnT = P.tile([128, 16], BF16)
        parg = P.tile([128, 2048], BF16)        # actx + h_att (pre-tanh)
        prep = P.tile([128, 2048], BF16)
        garg = P.tile([128, 64], F32)
        dmy = P.tile([128, 4], BF16)            # constant rhs for PE warm-up MMs
        tg = P.tile([128, 64], F32)             # tanh'd gates
        pq = P.tile([128, 32], F32)             # p | q
        tcs = P.tile([128, 16], F32)            # tanh(c)
        h2 = P.tile([128, 16], F32)
        bT = P.tile([128, 16], F32)
        batt = P.tile([128, 4], F32)
        w_sb = P.tile([128, 4], BF16)
        ctx_sb = P.tile([128, 2048], BF16)
        actx = P.tile([128, 2048], BF16)

        pp_pre = psumP.tile([128, 64], F32)
        # small attention tiles share one PSUM bank (bank-granular alloc)
        pp_att = psumP.tile([128, 64], F32)
        pp_hatt = pp_att[:, 0:16]
        pp_prjT = pp_att[:, 16:20]
        pp_sum = pp_att[:, 20:24]
        pp_wc = pp_att[:, 24:40]
        pp_trb = psumP.tile([128, 128], BF16)
        pp_dmy = psumP.tile([128, 4], F32)

        nc.vector.memset(hT[:, :], 0.0)
        nc.vector.memset(cT[:, :], 0.0)
        nc.vector.memset(onesb[:, :], 1.0)
        nc.vector.memset(dmy[:, :], 0.125)

        # natural-layout context load (l on partitions, contiguous rows)
        for b_ in range(B_LOC):
            nc.gpsimd.dma_start(ctx_sb[:, b_ * 512:(b_ + 1) * 512],
                                ctx_d[b_, :, :])

        # ---------------- pre-pass (freed afterwards) ----------------
        with tc.tile_pool(name="pre", bufs=1) as PRE, \
             tc.tile_pool(name="psum_pre", bufs=1, space="PSUM") as psumX:
            xT = PRE.tile([128, 4096], BF16)    # col = kt*1024 + b*256 + t
            x_nat = PRE.tile([128, 4096], F32)  # col = (b*2+th)*512 + d
            W_sb = PRE.tile([128, 8192], BF16)  # col = kt*2048 + m
            Wc_sb = PRE.tile([128, 2048], BF16)  # col = ct*512 + a
            ctxT = PRE.tile([128, 2048], BF16)  # col = b*512 + ct*128 + l

            nc.gpsimd.dma_start(idN[:, :], id_d[:, :])
            nc.vector.tensor_copy(idNb[:, :], idN[:, :])
            for b_ in range(B_LOC):
                for th in range(2):
                    nc.gpsimd.dma_start(
                        x_nat[:, (b_ * 2 + th) * 512:(b_ * 2 + th + 1) * 512],
                        x_d[b_, th * 128:(th + 1) * 128, :])
            # on-chip transpose of x: [t, d] blocks -> [d, t] (f32 -> bf16)
            for b_ in range(B_LOC):
                for th in range(2):
                    for kt in range(KT):
                        pt = psumX.tile([128, 128], F32, tag="pa")
                        nc.tensor.transpose(
                            pt[:, :],
                            x_nat[:, (b_ * 2 + th) * 512 + kt * 128:
                                  (b_ * 2 + th) * 512 + (kt + 1) * 128],
                            idN[:, :])
                        nc.vector.tensor_copy(
                            xT[:, kt * 1024 + b_ * 256 + th * 128:
                               kt * 1024 + b_ * 256 + (th + 1) * 128],
                            pt[:, :])
            for kt in range(KT):
                nc.gpsimd.dma_start(W_sb[:, kt * 2048:(kt + 1) * 2048],
                                    W_d[kt * 128:(kt + 1) * 128, :])
                nc.gpsimd.dma_start(Wc_sb[:, kt * 512:(kt + 1) * 512],
                                    Wc_d[kt * 128:(kt + 1) * 128, :])
            # transposed loads of small vectors via PE (row-major DMA + T)
            bt_nat = PRE.tile([16, 128], F32)
            nc.gpsimd.dma_start(bt_nat[:, :], b_d[:].rearrange("(m p) -> m p", p=128))
            pt = psumX.tile([128, 16], F32, tag="pa")
            nc.tensor.transpose(pt[:, :], bt_nat[:, :], idN[0:16, 0:16])
            nc.vector.tensor_copy(bT[:, :], pt[:, :])
            ba_nat = PRE.tile([4, 128], F32)
            nc.gpsimd.dma_start(ba_nat[:, :], ba_d[:].rearrange("(m p) -> m p", p=128))
            pt = psumX.tile([128, 16], F32, tag="pa")
            nc.tensor.transpose(pt[:, 0:4], ba_nat[:, :], idN[0:4, 0:4])
            nc.vector.tensor_copy(batt[:, :], pt[:, 0:4])
            wp_nat = PRE.tile([4, 128], F32)
            nc.gpsimd.dma_start(wp_nat[:, :],
                                wp_d[:, :].rearrange("(m p) one -> m (p one)", p=128))
            pt = psumX.tile([128, 16], F32, tag="pa")
            nc.tensor.transpose(pt[:, 0:4], wp_nat[:, :], idN[0:4, 0:4])
            nc.vector.tensor_copy(w_sb[:, :], pt[:, 0:4])
            # context transposed (c on partitions) via PE from ctx_sb
            for b_ in range(B_LOC):
                for ct in range(KT):
                    pcb = psumX.tile([128, 128], BF16, tag="pa")
                    nc.tensor.transpose(
                        pcb[:, :],
                        ctx_sb[:, b_ * 512 + ct * 128: b_ * 512 + (ct + 1) * 128],
                        idNb[:, :])
                    nc.vector.tensor_copy(
                        ctxT[:, b_ * 512 + ct * 128: b_ * 512 + (ct + 1) * 128],
                        pcb[:, :])

            # xW^T : per (mtile, b) accumulate over ktiles, N=256 (t)
            xWT3 = xWT[:, :].rearrange("p (t m) -> p t m", m=64)
            for mt in range(MT):
                for b_ in range(B_LOC):
                    px = psumX.tile([128, 256], F32, tag="px")
                    for kt in range(KT):
                        nc.tensor.matmul(
                            px[:, :],
                            lhsT=W_sb[:, kt * 2048 + mt * 128: kt * 2048 + (mt + 1) * 128],
                            rhs=xT[:, kt * 1024 + b_ * 256: kt * 1024 + (b_ + 1) * 256],
                            start=(kt == 0), stop=(kt == KT - 1))
                    # fold LSTM bias b while evacuating PSUM
                    nc.scalar.activation(
                        xWT3[:, :, mt * 4 + b_: mt * 4 + b_ + 1].squeeze(),
                        px[:, :], AF.Identity, bias=bT[:, mt:mt + 1])

            # att_ctx^T = Wctx^T @ ctx^T (+ b_att)
            for b_ in range(B_LOC):
                for at in range(KT):
                    pa = psumX.tile([128, 128], F32, tag="pa")
                    for ct in range(KT):
                        nc.tensor.matmul(
                            pa[:, :],
                            lhsT=Wc_sb[:, ct * 512 + at * 128: ct * 512 + (at + 1) * 128],
                            rhs=ctxT[:, b_ * 512 + ct * 128: b_ * 512 + (ct + 1) * 128],
                            start=(ct == 0), stop=(ct == KT - 1))
                    nc.scalar.activation(
                        actx[:, b_ * 512 + at * 128: b_ * 512 + (at + 1) * 128],
                        pa[:, :], AF.Identity, bias=batt[:, at:at + 1])

        # weights for the scan (allocated after pre-pass frees its space)
        WTS = ctx.enter_context(tc.tile_pool(name="wts", bufs=1))
        U_sb = WTS.tile([128, 8192], BF16)
        V_sb = WTS.tile([128, 8192], BF16)
        Wh_sb = WTS.tile([128, 2048], BF16)
        for kt in range(KT):
            nc.gpsimd.dma_start(U_sb[:, kt * 2048:(kt + 1) * 2048],
                                U_d[kt * 128:(kt + 1) * 128, :])
            nc.gpsimd.dma_start(V_sb[:, kt * 2048:(kt + 1) * 2048],
                                V_d[kt * 128:(kt + 1) * 128, :])
            nc.gpsimd.dma_start(Wh_sb[:, kt * 512:(kt + 1) * 512],
                                Wh_d[kt * 128:(kt + 1) * 128, :])

        # ---------------- the scan ----------------
        import os as _os
        UNROLL = int(_os.environ.get("KERNEL_UNROLL", "8"))
        DUMMY = int(_os.environ.get("KERNEL_DUMMY", "24"))
        TSCAN = int(_os.environ.get("KERNEL_TSCAN", str(T)))

        def u_chunk(kts):
            """U^T h matmuls for the given kt values (16 each)."""
            for kt in kts:
                rhs_h = hT[:, kt * 4:(kt + 1) * 4]
                for mt in range(MT):
                    nc.tensor.matmul(
                        pp_pre[:, mt * 4:(mt + 1) * 4],
                        lhsT=U_sb[:, kt * 2048 + mt * 128: kt * 2048 + (mt + 1) * 128],
                        rhs=rhs_h, start=(kt == 0 and mt == 0), stop=False,
                        skip_group_check=True)

        # broadcast views for the h_att bias add: actx[p, (b at l)] + hatt[p, (at b)]
        actx_v = actx[:, :].rearrange("p (b at l) -> p b at l", b=4, at=4, l=128)
        parg_v = parg[:, :].rearrange("p (b at l) -> p b at l", b=4, at=4, l=128)
        hatt_v = pp_hatt.rearrange("p (at b) -> p b at", at=4, b=4)
        # wcnT evac folds the softmax normalization: per-column scale r[b]
        wcn_v = wcnT[:, :].rearrange("p (ct b) -> p ct b", ct=4, b=4)
        ppwc_v = pp_wc.rearrange("p (ct b) -> p ct b", ct=4, b=4)
        r_v = r128[:, None, :].broadcast_to([128, 4, 4])

        def step_body(t, xoff=None, hoff=None):
            # --- PE: h_att^T = Wh^T h (16) ---
            for kt in range(KT):
                rhs_h = hT[:, kt * 4:(kt + 1) * 4]
                for ma in range(4):
                    nc.tensor.matmul(
                        pp_hatt[:, ma * 4:(ma + 1) * 4],
                        lhsT=Wh_sb[:, kt * 512 + ma * 128: kt * 512 + (ma + 1) * 128],
                        rhs=rhs_h, start=(kt == 0 and ma == 0),
                        stop=(kt == KT - 1 and ma == 3), skip_group_check=True)

            # --- DVE: parg = actx + h_att (broadcast add, all-bf16 for 2x DVE)
            # --- ACT: prep = tanh(parg) (2 wide ops) ---
            for half in range(2):
                bs = slice(2 * half, 2 * half + 2)
                nc.vector.tensor_tensor(
                    parg_v[:, bs], actx_v[:, bs],
                    hatt_v[:, bs, :, None].broadcast_to([128, 2, 4, 128]),
                    ALU.add)
                nc.scalar.activation(prep[:, 1024 * half:1024 * (half + 1)],
                                     parg[:, 1024 * half:1024 * (half + 1)],
                                     AF.Tanh)

            # --- PE: U chunk A fills the prep window ---
            u_chunk((0, 1))

            # --- PE: prj^T[l, b] = sum_a prep[a, l] w[a] (16, N=1),
            # split so b0/b1 fire after the first tanh half ---
            def prj(b_):
                for at in range(KT):
                    nc.tensor.matmul(
                        pp_prjT[:, b_:b_ + 1],
                        lhsT=prep[:, b_ * 512 + at * 128: b_ * 512 + (at + 1) * 128],
                        rhs=w_sb[:, at:at + 1],
                        start=(at == 0), stop=(at == KT - 1),
                        skip_group_check=True)
            prj(0); prj(1)
            u_chunk((2,))
            prj(2); prj(3)
            u_chunk((3,))

            # --- ACT: exp (no max-subtraction; inputs bounded) ---
            nc.scalar.activation(expT[:, :], pp_prjT[:, :], AF.Exp)
            # --- PE: row sums on every partition via all-ones stationary ---
            nc.tensor.matmul(pp_sum[:, :], lhsT=onesb[:, :], rhs=expT[:, :],
                             start=True, stop=True, skip_group_check=True)
            # --- DVE: r = 1/sums (runs while wctx matmuls stream) ---
            nc.vector.reciprocal(r128[:, :], pp_sum[:, :])

            # --- PE: unnormalized wctx^T[c, b] from expT (ct-major), with the
            # softmax 1/sum folded into per-ct PSUM evacuations so V(kt) can
            # start as soon as its wcnT chunk lands ---
            for ct in range(KT):
                for b_ in range(B_LOC):
                    nc.tensor.matmul(
                        pp_wc[:, ct * 4 + b_: ct * 4 + b_ + 1],
                        lhsT=ctx_sb[:, b_ * 512 + ct * 128: b_ * 512 + (ct + 1) * 128],
                        rhs=expT[:, b_:b_ + 1],
                        start=True, stop=True, skip_group_check=True)
            for ct in range(KT):
                nc.vector.tensor_tensor(wcn_v[:, ct], ppwc_v[:, ct],
                                        r_v[:, ct], ALU.mult)

            # --- PE: preact^T += V^T wctx (64) ---
            for kt in range(KT):
                rhs_w = wcnT[:, kt * 4:(kt + 1) * 4]
                for mt in range(MT):
                    nc.tensor.matmul(
                        pp_pre[:, mt * 4:(mt + 1) * 4],
                        lhsT=V_sb[:, kt * 2048 + mt * 128: kt * 2048 + (mt + 1) * 128],
                        rhs=rhs_w,
                        start=False, stop=(kt == KT - 1 and mt == MT - 1),
                        skip_group_check=True)

            # --- gates. xWT i/f/o region is prescaled (0.5x + 0.5b), so the
            # whole gate stream needs one stt + one add + ONE tanh.
            # PE warm-up filler: keeps the clock up through the gates window
            for d_ in range(DUMMY):
                nc.tensor.matmul(pp_dmy[:, :],
                                 lhsT=U_sb[:, (d_ % 64) * 128:(d_ % 64) * 128 + 128],
                                 rhs=dmy[:, :], start=True, stop=True,
                                 skip_group_check=True)
            xo = t * 64 if xoff is None else xoff
            nc.vector.tensor_add(garg[:, :], pp_pre[:, :], xWT[:, bass.ds(xo, 64)])
            nc.scalar.activation(tg[:, 0:48], garg[:, 0:48], AF.Tanh, scale=0.5)
            nc.scalar.activation(tg[:, 48:64], garg[:, 48:64], AF.Tanh)
            # Doubled state: cT holds 2c, hT holds 2h (U and W_h_att are
            # pre-halved host-side; the epilogue halves the history).
            #   2c' = 0.5*(tf+1)*(2c) + (ti+1)*cand
            #   2h' = (to+1)*tanh(0.5*(2c'))
            nc.vector.scalar_tensor_tensor(pq[:, 0:16], tg[:, 16:32], 1.0,
                                           cT[:, :], ALU.add, ALU.mult)
            nc.vector.scalar_tensor_tensor(pq[:, 16:32], tg[:, 0:16], 1.0,
                                           tg[:, 48:64], ALU.add, ALU.mult)
            nc.vector.scalar_tensor_tensor(cT[:, :], pq[:, 0:16], 0.5,
                                           pq[:, 16:32], ALU.mult, ALU.add)
            nc.scalar.activation(tcs[:, :], cT[:, :], AF.Tanh, scale=0.5)
            nc.vector.scalar_tensor_tensor(hT[:, :], tg[:, 32:48], 1.0,
                                           tcs[:, :], ALU.add, ALU.mult)

            # --- store h_t into the SBUF history buffer (Pool engine) ---
            hsT4 = hsT[:, :].rearrange("p (k b t) -> p k b t", b=4, t=T)
            nc.gpsimd.tensor_copy(hsT4[:, :, :, bass.ds(t if hoff is None else hoff, 1)].squeeze(), hT[:, :])

        with tc.For_i(0, repeat, 1) as _r, \
             tc.For_i(0, TSCAN // UNROLL, 1) as tb:
            base_x = nc.vector.snap(tb * (UNROLL * 64))
            base_h = nc.gpsimd.snap(tb * UNROLL)
            for u in range(UNROLL):
                step_body(tb * UNROLL + u,
                          xoff=base_x + u * 64, hoff=base_h + u)

        # ---------------- epilogue: transpose h history, store ----------------
        STG = ctx.enter_context(tc.tile_pool(name="stage", bufs=2))
        for b_ in range(B_LOC):
            for th in range(2):
                st = STG.tile([128, 512], F32, tag="st")
                for kt in range(KT):
                    nc.tensor.transpose(
                        pp_trb[:, :],
                        hsT[:, kt * 1024 + b_ * 256 + th * 128:
                            kt * 1024 + b_ * 256 + (th + 1) * 128],
                        idNb[:, :])
                    # halve: hsT holds 2h (doubled-state trick)
                    nc.vector.tensor_scalar_mul(st[:, kt * 128:(kt + 1) * 128],
                                                pp_trb[:, :], 0.5)
                nc.sync.dma_start(hs_d[b_, th * 128:(th + 1) * 128, :], st[:, :])

    split_multi_waits(nc)
    return nc


_NC_CACHE = {}


def _get_nc(repeat=1):
    if repeat not in _NC_CACHE:
        _NC_CACHE[repeat] = build_nc(repeat)
    return _NC_CACHE[repeat]


def kernel(x, context, W, V, U, b, W_h_att, W_ctx_att, b_att, w_att_prj,
           bench_repeat=1, **run_kwargs):
    import ml_dtypes
    BF = ml_dtypes.bfloat16
    nc = _get_nc(bench_repeat)
    f32 = lambda a: np.ascontiguousarray(np.asarray(a), dtype=np.float32)
    bf16 = lambda a: np.ascontiguousarray(np.asarray(a, dtype=np.float32).astype(BF))
    x, context = f32(x), np.asarray(context, dtype=np.float32)
    # U and W_h_att pre-halved: the device state tiles hold 2h/2c
    shared = dict(W=bf16(W), V=bf16(V), U=bf16(np.asarray(U) * 0.5), b=f32(b),
                  W_h_att=bf16(np.asarray(W_h_att) * 0.5), W_ctx_att=bf16(W_ctx_att),
                  b_att=f32(b_att), w_att_prj=f32(w_att_prj),
                  ident=np.eye(128, dtype=np.float32))
    in_maps = []
    for c in range(NCORES):
        m = dict(shared)
        m["x"] = np.ascontiguousarray(x[c * B_LOC:(c + 1) * B_LOC])
        m["context"] = bf16(context[c * B_LOC:(c + 1) * B_LOC])
        in_maps.append(m)
    res = run_bass_kernel_spmd(nc, in_maps, core_ids=list(range(NCORES)),
                               **run_kwargs)
    out = np.concatenate([r["hs"] for r in res.results], axis=0)
    kernel.last_result = res
    return out


if __name__ == "__main__":
    rng = np.random.default_rng(0)
    ins = {
        "x": rng.standard_normal((32, T, DIN), dtype=np.float32),
        "context": rng.standard_normal((32, L, C), dtype=np.float32),
        "W": (rng.standard_normal((DIN, 4 * D), dtype=np.float32) * 0.05),
        "V": (rng.standard_normal((C, 4 * D), dtype=np.float32) * 0.05),
        "U": (rng.standard_normal((D, 4 * D), dtype=np.float32) * 0.05),
        "b": np.zeros(4 * D, np.float32),
        "W_h_att": (rng.standard_normal((D, A), dtype=np.float32) * 0.05),
        "W_ctx_att": (rng.standard_normal((C, A), dtype=np.float32) * 0.05),
        "b_att": np.zeros(A, np.float32),
        "w_att_prj": (rng.standard_normal((A, 1), dtype=np.float32) * 0.05),
    }
    out = kernel(**ins)
    print("out", out.shape, out.dtype, float(np.abs(out).max()))


# revision 31
# speedup vs baseline: 4.0318x; 1.1010x over previous
"""AttentionLSTM Trainium2 kernel (bf16 matmuls + restructured softmax).

Sharding: data-parallel over batch. B=32 across 8 cores -> B_local=4 per
core; weights replicated; context/att-context shard with batch.

Per-core layout (all "transposed": feature dim on partitions):
  hT      bf16 [128, 16]  col = kt*4 + b   (d = kt*128 + p)
  cT      f32  [128, 16]
  preact.T accumulates in PSUM f32 [128, 64]  col = mt*4 + b (n = mt*128+p)
  xWT     bf16 [128, T*64] col = t*64 + mt*4 + b  (x@W + b, precomputed)
  actx    f32  [128, 2048] col = b*512 + at*128 + l  (a on partitions)
  prep    bf16 [128, 2048] tanh(actx + h_att), same layout
  ctx_sb  bf16 [128, 2048] col = b*512 + c          (l on partitions)
  hsT     bf16 [128, T*16] col = kt*1024 + b*256 + t (h history)

All matmul operands are bf16 (1 PE cycle/row vs 4 for fp32); PSUM
accumulation stays fp32.  Softmax runs in the transposed domain:
prj^T [l=128, b] via prep-as-stationary matmuls, exp on [128,4], row
sums via an all-ones matmul (same value on every partition), so no
PE transposes of alpha and no [1,512]-wide ops on the critical path.

sigmoid(x) = 0.5 + 0.5*tanh(x/2) so the whole kernel only needs the
exp_and_others ACT table set (exp + tanh), loaded once.
Softmax skips max-subtraction: |prj| <= sum|w_att| ~ 20, exp is safe.

PE program order per step is staged so the tensor engine always has
ready work while ACT/DVE run the attention chain:
  Wh | U(kt0,1) | prj(b01) | U(kt2) | prj(b23) | U(kt3) | sum | wctx | V

State is kept doubled (hT=2h, cT=2c; U and W_h_att pre-halved on the
host, history halved in the epilogue) so the whole gate tail is five
DVE/ACT ops; the h_att bias-add reads its PSUM tile directly via a
stride-0 broadcast view, and softmax normalization is folded into the
per-ct wctx PSUM evacuations.
"""

import numpy as np
from contextlib import ExitStack

import concourse.bass as bass
import concourse.mybir as mybir
import concourse.tile as tile
from concourse.bass_utils import run_bass_kernel_spmd

F32 = mybir.dt.float32
BF16 = mybir.dt.bfloat16
AF = mybir.ActivationFunctionType
ALU = mybir.AluOpType

B_LOC, T, DIN, D, C, A, L = 4, 256, 512, 512, 512, 512, 128
KT = 4          # 512/128 k-tiles
MT = 16         # 2048/128 m-tiles of the gate dim
NCORES = 8


def split_multi_waits(nc, max_waits=1):
    """This walrus build rejects >1 sync-wait per instruction on some
    opcodes. Hoist extra waits into standalone EventSemaphore preludes."""
    ctr = [0]
    n_fixed = 0

    def fix_block(blk):
        nonlocal n_fixed
        new_insts = []
        for inst in blk.instructions:
            si = inst.sync_info
            waits = list(si.on_wait) if si is not None else []
            if len(waits) > max_waits:
                for w in waits[:-max_waits]:
                    ctr[0] += 1
                    new_insts.append(mybir.InstEventSemaphore(
                        name=f"I-waitsplit-{ctr[0]}",
                        engine=inst.engine, ins=[], outs=[],
                        sync_info=mybir.SyncInfo(on_wait=[w], on_update=[]),
                    ))
                si.on_wait = waits[-max_waits:]
                n_fixed += 1
            new_insts.append(inst)
        blk.instructions[:] = new_insts

    for f in nc.m.functions:
        for blk in f.blocks:
            fix_block(blk)
    return n_fixed


def build_nc(repeat=1):
    nc = bass.Bass()
    x_d = nc.dram_tensor("x", [B_LOC, T, DIN], F32, kind="ExternalInput")
    ctx_d = nc.dram_tensor("context", [B_LOC, L, C], BF16, kind="ExternalInput")
    W_d = nc.dram_tensor("W", [DIN, 4 * D], BF16, kind="ExternalInput")
    V_d = nc.dram_tensor("V", [C, 4 * D], BF16, kind="ExternalInput")
    U_d = nc.dram_tensor("U", [D, 4 * D], BF16, kind="ExternalInput")
    b_d = nc.dram_tensor("b", [4 * D], F32, kind="ExternalInput")
    Wh_d = nc.dram_tensor("W_h_att", [D, A], BF16, kind="ExternalInput")
    Wc_d = nc.dram_tensor("W_ctx_att", [C, A], BF16, kind="ExternalInput")
    ba_d = nc.dram_tensor("b_att", [A], F32, kind="ExternalInput")
    wp_d = nc.dram_tensor("w_att_prj", [A, 1], F32, kind="ExternalInput")
    id_d = nc.dram_tensor("ident", [128, 128], F32, kind="ExternalInput")
    hs_d = nc.dram_tensor("hs", [B_LOC, T, D], F32, kind="ExternalOutput")

    with ExitStack() as ctx:
        tc = ctx.enter_context(tile.TileContext(nc))
        P = ctx.enter_context(tc.tile_pool(name="persist", bufs=1))
        psumP = ctx.enter_context(tc.tile_pool(name="psumP", bufs=1, space="PSUM"))

        # ---------------- persistent tiles ----------------
        xWT = P.tile([128, T * 64], BF16)       # 32KB/part (PE-folded into PSUM)
        hsT = P.tile([128, T * 16], BF16)       # all h_t, col = kt*1024+b*256+t
        idN = P.tile([128, 128], F32)
        idNb = P.tile([128, 128], BF16)
        onesb = P.tile([128, 128], BF16)
        hT = P.tile([128, 16], BF16)            # stores 2*h (see doubled-state note)
        cT = P.tile([128, 16], F32)             # stores 2*c
        expT = P.tile([128, 4], BF16)
        r128 = P.tile([128, 4], F32)
        wcnT = P.tile([128, 16], BF16)
        parg = P.tile([128, 2048], BF16)        # actx + h_att (pre-tanh)
        prep = P.tile([128, 2048], BF16)
        garg = P.tile([128, 64], F32)
        dmy = P.tile([128, 4], BF16)            # constant rhs for PE warm-up MMs
        tg = P.tile([128, 64], F32)             # tanh'd gates
        pq = P.tile([128, 32], F32)             # p | q
        tcs = P.tile([128, 16], F32)            # tanh(c)
        bT = P.tile([128, 16], F32)
        batt = P.tile([128, 4], F32)
        w_sb = P.tile([128, 4], BF16)
        ctx_sb = P.tile([128, 2048], BF16)
        actx = P.tile([128, 2048], BF16)

        pp_pre = psumP.tile([128, 64], F32)
        # small attention tiles share one PSUM bank (bank-granular alloc)
        pp_att = psumP.tile([128, 64], F32)
        pp_hatt = pp_att[:, 0:16]
        pp_prjT = pp_att[:, 16:20]
        pp_sum = pp_att[:, 20:24]
        pp_wc = pp_att[:, 24:40]
        pp_trb = psumP.tile([128, 128], BF16)
        pp_dmy = psumP.tile([128, 4], F32)

        nc.vector.memset(hT[:, :], 0.0)
        nc.vector.memset(cT[:, :], 0.0)
        nc.vector.memset(onesb[:, :], 1.0)
        nc.vector.memset(dmy[:, :], 0.125)

        # natural-layout context load (l on partitions, contiguous rows)
        for b_ in range(B_LOC):
            nc.gpsimd.dma_start(ctx_sb[:, b_ * 512:(b_ + 1) * 512],
                                ctx_d[b_, :, :])

        # ---------------- pre-pass (freed afterwards) ----------------
        with tc.tile_pool(name="pre", bufs=1) as PRE, \
             tc.tile_pool(name="psum_pre", bufs=1, space="PSUM") as psumX:
            xT = PRE.tile([128, 4096], BF16)    # col = kt*1024 + b*256 + t
            x_nat = PRE.tile([128, 4096], F32)  # col = (b*2+th)*512 + d
            W_sb = PRE.tile([128, 8192], BF16)  # col = kt*2048 + m
            Wc_sb = PRE.tile([128, 2048], BF16)  # col = ct*512 + a
            ctxT = PRE.tile([128, 2048], BF16)  # col = b*512 + ct*128 + l

            nc.gpsimd.dma_start(idN[:, :], id_d[:, :])
            nc.vector.tensor_copy(idNb[:, :], idN[:, :])
            for b_ in range(B_LOC):
                for th in range(2):
                    nc.gpsimd.dma_start(
                        x_nat[:, (b_ * 2 + th) * 512:(b_ * 2 + th + 1) * 512],
                        x_d[b_, th * 128:(th + 1) * 128, :])
            # on-chip transpose of x: [t, d] blocks -> [d, t] (f32 -> bf16)
            for b_ in range(B_LOC):
                for th in range(2):
                    for kt in range(KT):
                        pt = psumX.tile([128, 128], F32, tag="pa")
                        nc.tensor.transpose(
                            pt[:, :],
                            x_nat[:, (b_ * 2 + th) * 512 + kt * 128:
                                  (b_ * 2 + th) * 512 + (kt + 1) * 128],
                            idN[:, :])
                        nc.vector.tensor_copy(
                            xT[:, kt * 1024 + b_ * 256 + th * 128:
                               kt * 1024 + b_ * 256 + (th + 1) * 128],
                            pt[:, :])
            for kt in range(KT):
                nc.gpsimd.dma_start(W_sb[:, kt * 2048:(kt + 1) * 2048],
                                    W_d[kt * 128:(kt + 1) * 128, :])
                nc.gpsimd.dma_start(Wc_sb[:, kt * 512:(kt + 1) * 512],
                                    Wc_d[kt * 128:(kt + 1) * 128, :])
            # transposed loads of small vectors via PE (row-major DMA + T)
            bt_nat = PRE.tile([16, 128], F32)
            nc.gpsimd.dma_start(bt_nat[:, :], b_d[:].rearrange("(m p) -> m p", p=128))
            pt = psumX.tile([128, 16], F32, tag="pa")
            nc.tensor.transpose(pt[:, :], bt_nat[:, :], idN[0:16, 0:16])
            nc.vector.tensor_copy(bT[:, :], pt[:, :])
            ba_nat = PRE.tile([4, 128], F32)
            nc.gpsimd.dma_start(ba_nat[:, :], ba_d[:].rearrange("(m p) -> m p", p=128))
            pt = psumX.tile([128, 16], F32, tag="pa")
            nc.tensor.transpose(pt[:, 0:4], ba_nat[:, :], idN[0:4, 0:4])
            nc.vector.tensor_copy(batt[:, :], pt[:, 0:4])
            wp_nat = PRE.tile([4, 128], F32)
            nc.gpsimd.dma_start(wp_nat[:, :],
                                wp_d[:, :].rearrange("(m p) one -> m (p one)", p=128))
            pt = psumX.tile([128, 16], F32, tag="pa")
            nc.tensor.transpose(pt[:, 0:4], wp_nat[:, :], idN[0:4, 0:4])
            nc.vector.tensor_copy(w_sb[:, :], pt[:, 0:4])
            # context transposed (c on partitions) via PE from ctx_sb
            for b_ in range(B_LOC):
                for ct in range(KT):
                    pcb = psumX.tile([128, 128], BF16, tag="pa")
                    nc.tensor.transpose(
                        pcb[:, :],
                        ctx_sb[:, b_ * 512 + ct * 128: b_ * 512 + (ct + 1) * 128],
                        idNb[:, :])
                    nc.vector.tensor_copy(
                        ctxT[:, b_ * 512 + ct * 128: b_ * 512 + (ct + 1) * 128],
                        pcb[:, :])

            # xW^T : per (mtile, b) accumulate over ktiles, N=256 (t)
            xWT3 = xWT[:, :].rearrange("p (t m) -> p t m", m=64)
            for mt in range(MT):
                for b_ in range(B_LOC):
                    px = psumX.tile([128, 256], F32, tag="px")
                    for kt in range(KT):
                        nc.tensor.matmul(
                            px[:, :],
                            lhsT=W_sb[:, kt * 2048 + mt * 128: kt * 2048 + (mt + 1) * 128],
                            rhs=xT[:, kt * 1024 + b_ * 256: kt * 1024 + (b_ + 1) * 256],
                            start=(kt == 0), stop=(kt == KT - 1))
                    # fold LSTM bias b while evacuating PSUM
                    nc.scalar.activation(
                        xWT3[:, :, mt * 4 + b_: mt * 4 + b_ + 1].squeeze(),
                        px[:, :], AF.Identity, bias=bT[:, mt:mt + 1])

            # att_ctx^T = Wctx^T @ ctx^T (+ b_att)
            for b_ in range(B_LOC):
                for at in range(KT):
                    pa = psumX.tile([128, 128], F32, tag="pa")
                    for ct in range(KT):
                        nc.tensor.matmul(
                            pa[:, :],
                            lhsT=Wc_sb[:, ct * 512 + at * 128: ct * 512 + (at + 1) * 128],
                            rhs=ctxT[:, b_ * 512 + ct * 128: b_ * 512 + (ct + 1) * 128],
                            start=(ct == 0), stop=(ct == KT - 1))
                    nc.scalar.activation(
                        actx[:, b_ * 512 + at * 128: b_ * 512 + (at + 1) * 128],
                        pa[:, :], AF.Identity, bias=batt[:, at:at + 1])

        # weights for the scan (allocated after pre-pass frees its space)
        WTS = ctx.enter_context(tc.tile_pool(name="wts", bufs=1))
        U_sb = WTS.tile([128, 8192], BF16)
        V_sb = WTS.tile([128, 8192], BF16)
        Wh_sb = WTS.tile([128, 2048], BF16)
        for kt in range(KT):
            nc.gpsimd.dma_start(U_sb[:, kt * 2048:(kt + 1) * 2048],
                                U_d[kt * 128:(kt + 1) * 128, :])
            nc.gpsimd.dma_start(V_sb[:, kt * 2048:(kt + 1) * 2048],
                                V_d[kt * 128:(kt + 1) * 128, :])
            nc.gpsimd.dma_start(Wh_sb[:, kt * 512:(kt + 1) * 512],
                                Wh_d[kt * 128:(kt + 1) * 128, :])

        # ---------------- the scan ----------------
        import os as _os
        UNROLL = int(_os.environ.get("KERNEL_UNROLL", "8"))
        DUMMY = int(_os.environ.get("KERNEL_DUMMY", "24"))
        TSCAN = int(_os.environ.get("KERNEL_TSCAN", str(T)))

        def u_chunk(kts):
            """U^T h matmuls for the given kt values (16 each)."""
            for kt in kts:
                rhs_h = hT[:, kt * 4:(kt + 1) * 4]
                for mt in range(MT):
                    nc.tensor.matmul(
                        pp_pre[:, mt * 4:(mt + 1) * 4],
                        lhsT=U_sb[:, kt * 2048 + mt * 128: kt * 2048 + (mt + 1) * 128],
                        rhs=rhs_h, start=(kt == 0 and mt == 0), stop=False,
                        skip_group_check=True)

        # broadcast views for the h_att bias add: actx[p, (b at l)] + hatt[p, (at b)]
        actx_v = actx[:, :].rearrange("p (b at l) -> p b at l", b=4, at=4, l=128)
        parg_v = parg[:, :].rearrange("p (b at l) -> p b at l", b=4, at=4, l=128)
        hatt_v = pp_hatt.rearrange("p (at b) -> p b at", at=4, b=4)
        # wcnT evac folds the softmax normalization: per-column scale r[b]
        wcn_v = wcnT[:, :].rearrange("p (ct b) -> p ct b", ct=4, b=4)
        ppwc_v = pp_wc.rearrange("p (ct b) -> p ct b", ct=4, b=4)
        r_v = r128[:, None, :].broadcast_to([128, 4, 4])

        def step_body(t, xoff=None, hoff=None):
            # --- PE: h_att^T = Wh^T h (16) ---
            for kt in range(KT):
                rhs_h = hT[:, kt * 4:(kt + 1) * 4]
                for ma in range(4):
                    nc.tensor.matmul(
                        pp_hatt[:, ma * 4:(ma + 1) * 4],
                        lhsT=Wh_sb[:, kt * 512 + ma * 128: kt * 512 + (ma + 1) * 128],
                        rhs=rhs_h, start=(kt == 0 and ma == 0),
                        stop=(kt == KT - 1 and ma == 3), skip_group_check=True)

            # --- DVE: parg = actx + h_att (broadcast add, all-bf16 for 2x DVE)
            # --- ACT: prep = tanh(parg) (2 wide ops) ---
            for half in range(2):
                bs = slice(2 * half, 2 * half + 2)
                nc.vector.tensor_tensor(
                    parg_v[:, bs], actx_v[:, bs],
                    hatt_v[:, bs, :, None].broadcast_to([128, 2, 4, 128]),
                    ALU.add)
                nc.scalar.activation(prep[:, 1024 * half:1024 * (half + 1)],
                                     parg[:, 1024 * half:1024 * (half + 1)],
                                     AF.Tanh)

            # --- PE: U chunk A fills the prep window ---
            u_chunk((0, 1))

            # --- PE: prj^T[l, b] = sum_a prep[a, l] w[a] (16, N=1),
            # split so b0/b1 fire after the first tanh half ---
            def prj(b_):
                for at in range(KT):
                    nc.tensor.matmul(
                        pp_prjT[:, b_:b_ + 1],
                        lhsT=prep[:, b_ * 512 + at * 128: b_ * 512 + (at + 1) * 128],
                        rhs=w_sb[:, at:at + 1],
                        start=(at == 0), stop=(at == KT - 1),
                        skip_group_check=True)
            prj(0); prj(1)
            u_chunk((2,))
            prj(2); prj(3)
            u_chunk((3,))

            # --- ACT: exp (no max-subtraction; inputs bounded) ---
            nc.scalar.activation(expT[:, :], pp_prjT[:, :], AF.Exp)
            # --- PE: row sums on every partition via all-ones stationary ---
            nc.tensor.matmul(pp_sum[:, :], lhsT=onesb[:, :], rhs=expT[:, :],
                             start=True, stop=True, skip_group_check=True)
            # --- DVE: r = 1/sums (runs while wctx matmuls stream) ---
            nc.vector.reciprocal(r128[:, :], pp_sum[:, :])

            # --- PE: unnormalized wctx^T[c, b] from expT (ct-major), with the
            # softmax 1/sum folded into per-ct PSUM evacuations so V(kt) can
            # start as soon as its wcnT chunk lands ---
            for ct in range(KT):
                for b_ in range(B_LOC):
                    nc.tensor.matmul(
                        pp_wc[:, ct * 4 + b_: ct * 4 + b_ + 1],
                        lhsT=ctx_sb[:, b_ * 512 + ct * 128: b_ * 512 + (ct + 1) * 128],
                        rhs=expT[:, b_:b_ + 1],
                        start=True, stop=True, skip_group_check=True)
            for ct in range(KT):
                nc.vector.tensor_tensor(wcn_v[:, ct], ppwc_v[:, ct],
                                        r_v[:, ct], ALU.mult)

            # --- PE: preact^T += V^T wctx (64) ---
            for kt in range(KT):
                rhs_w = wcnT[:, kt * 4:(kt + 1) * 4]
                for mt in range(MT):
                    nc.tensor.matmul(
                        pp_pre[:, mt * 4:(mt + 1) * 4],
                        lhsT=V_sb[:, kt * 2048 + mt * 128: kt * 2048 + (mt + 1) * 128],
                        rhs=rhs_w,
                        start=False, stop=(kt == KT - 1 and mt == MT - 1),
                        skip_group_check=True)

            # --- gates. xWT i/f/o region is prescaled (0.5x + 0.5b), so the
            # whole gate stream needs one stt + one add + ONE tanh.
            # PE warm-up filler: keeps the clock up through the gates window
            for d_ in range(DUMMY):
                nc.tensor.matmul(pp_dmy[:, :],
                                 lhsT=U_sb[:, (d_ % 64) * 128:(d_ % 64) * 128 + 128],
                                 rhs=dmy[:, :], start=True, stop=True,
                                 skip_group_check=True)
            xo = t * 64 if xoff is None else xoff
            nc.vector.tensor_add(garg[:, :], pp_pre[:, :], xWT[:, bass.ds(xo, 64)])
            nc.scalar.activation(tg[:, 0:48], garg[:, 0:48], AF.Tanh, scale=0.5)
            nc.scalar.activation(tg[:, 48:64], garg[:, 48:64], AF.Tanh)
            # Doubled state: cT holds 2c, hT holds 2h (U and W_h_att are
            # pre-halved host-side; the epilogue halves the history).
            #   2c' = 0.5*(tf+1)*(2c) + (ti+1)*cand
            #   2h' = (to+1)*tanh(0.5*(2c'))
            nc.vector.scalar_tensor_tensor(pq[:, 0:16], tg[:, 16:32], 1.0,
                                           cT[:, :], ALU.add, ALU.mult)
            nc.vector.scalar_tensor_tensor(pq[:, 16:32], tg[:, 0:16], 1.0,
                                           tg[:, 48:64], ALU.add, ALU.mult)
            nc.vector.scalar_tensor_tensor(cT[:, :], pq[:, 0:16], 0.5,
                                           pq[:, 16:32], ALU.mult, ALU.add)
            nc.scalar.activation(tcs[:, :], cT[:, :], AF.Tanh, scale=0.5)
            nc.vector.scalar_tensor_tensor(hT[:, :], tg[:, 32:48], 1.0,
                                           tcs[:, :], ALU.add, ALU.mult)

            # --- store h_t into the SBUF history buffer (Pool engine) ---
            hsT4 = hsT[:, :].rearrange("p (k b t) -> p k b t", b=4, t=T)
            nc.gpsimd.tensor_copy(hsT4[:, :, :, bass.ds(t if hoff is None else hoff, 1)].squeeze(), hT[:, :])

        with tc.For_i(0, repeat, 1) as _r, \
             tc.For_i(0, TSCAN // UNROLL, 1) as tb:
            base_x = nc.vector.snap(tb * (UNROLL * 64))
            base_h = nc.gpsimd.snap(tb * UNROLL)
            for u in range(UNROLL):
                step_body(tb * UNROLL + u,
                          xoff=base_x + u * 64, hoff=base_h + u)

        # ---------------- epilogue: transpose h history, store ----------------
        STG = ctx.enter_context(tc.tile_pool(name="stage", bufs=2))
        for b_ in range(B_LOC):
            for th in range(2):
                st = STG.tile([128, 512], F32, tag="st")
                for kt in range(KT):
                    nc.tensor.transpose(
                        pp_trb[:, :],
                        hsT[:, kt * 1024 + b_ * 256 + th * 128:
                            kt * 1024 + b_ * 256 + (th + 1) * 128],
                        idNb[:, :])
                    # halve: hsT holds 2h (doubled-state trick)
                    nc.vector.tensor_scalar_mul(st[:, kt * 128:(kt + 1) * 128],
                                                pp_trb[:, :], 0.5)
                nc.sync.dma_start(hs_d[b_, th * 128:(th + 1) * 128, :], st[:, :])

    split_multi_waits(nc)
    return nc


_NC_CACHE = {}


def _get_nc(repeat=1):
    if repeat not in _NC_CACHE:
        _NC_CACHE[repeat] = build_nc(repeat)
    return _NC_CACHE[repeat]


def kernel(x, context, W, V, U, b, W_h_att, W_ctx_att, b_att, w_att_prj,
           bench_repeat=1, **run_kwargs):
    import ml_dtypes
    BF = ml_dtypes.bfloat16
    nc = _get_nc(bench_repeat)
    f32 = lambda a: np.ascontiguousarray(np.asarray(a), dtype=np.float32)
    bf16 = lambda a: np.ascontiguousarray(np.asarray(a, dtype=np.float32).astype(BF))
    x, context = f32(x), np.asarray(context, dtype=np.float32)
    # U and W_h_att pre-halved: the device state tiles hold 2h/2c
    shared = dict(W=bf16(W), V=bf16(V), U=bf16(np.asarray(U) * 0.5), b=f32(b),
                  W_h_att=bf16(np.asarray(W_h_att) * 0.5), W_ctx_att=bf16(W_ctx_att),
                  b_att=f32(b_att), w_att_prj=f32(w_att_prj),
                  ident=np.eye(128, dtype=np.float32))
    in_maps = []
    for c in range(NCORES):
        m = dict(shared)
        m["x"] = np.ascontiguousarray(x[c * B_LOC:(c + 1) * B_LOC])
        m["context"] = bf16(context[c * B_LOC:(c + 1) * B_LOC])
        in_maps.append(m)
    res = run_bass_kernel_spmd(nc, in_maps, core_ids=list(range(NCORES)),
                               **run_kwargs)
    out = np.concatenate([r["hs"] for r in res.results], axis=0)
    kernel.last_result = res
    return out


if __name__ == "__main__":
    rng = np.random.default_rng(0)
    ins = {
        "x": rng.standard_normal((32, T, DIN), dtype=np.float32),
        "context": rng.standard_normal((32, L, C), dtype=np.float32),
        "W": (rng.standard_normal((DIN, 4 * D), dtype=np.float32) * 0.05),
        "V": (rng.standard_normal((C, 4 * D), dtype=np.float32) * 0.05),
        "U": (rng.standard_normal((D, 4 * D), dtype=np.float32) * 0.05),
        "b": np.zeros(4 * D, np.float32),
        "W_h_att": (rng.standard_normal((D, A), dtype=np.float32) * 0.05),
        "W_ctx_att": (rng.standard_normal((C, A), dtype=np.float32) * 0.05),
        "b_att": np.zeros(A, np.float32),
        "w_att_prj": (rng.standard_normal((A, 1), dtype=np.float32) * 0.05),
    }
    out = kernel(**ins)
    print("out", out.shape, out.dtype, float(np.abs(out).max()))


# revision 32
# speedup vs baseline: 6.3340x; 1.5710x over previous
"""AttentionLSTM Trainium2 kernel (bf16 matmuls + restructured softmax).

Sharding: data-parallel over batch. B=32 across 8 cores -> B_local=4 per
core; weights replicated; context/att-context shard with batch.

Per-core layout (all "transposed": feature dim on partitions):
  hT      bf16 [128, 16]  col = kt*4 + b   (d = kt*128 + p)
  cT      f32  [128, 16]
  preact.T accumulates in PSUM f32 [128, 64]  col = mt*4 + b (n = mt*128+p)
  xWT     bf16 [128, T*64] col = t*64 + mt*4 + b  (x@W + b, precomputed)
  actx    f32  [128, 2048] col = b*512 + at*128 + l  (a on partitions)
  prep    bf16 [128, 2048] tanh(actx + h_att), same layout
  ctx_sb  bf16 [128, 2048] col = b*512 + c          (l on partitions)
  hsT     bf16 [128, T*16] col = kt*1024 + b*256 + t (h history)

All matmul operands are bf16 (1 PE cycle/row vs 4 for fp32); PSUM
accumulation stays fp32.  Softmax runs in the transposed domain:
prj^T [l=128, b] via prep-as-stationary matmuls, exp on [128,4], row
sums via an all-ones matmul (same value on every partition), so no
PE transposes of alpha and no [1,512]-wide ops on the critical path.

sigmoid(x) = 0.5 + 0.5*tanh(x/2) so the whole kernel only needs the
exp_and_others ACT table set (exp + tanh), loaded once.
Softmax skips max-subtraction: |prj| <= sum|w_att| ~ 20, exp is safe.

PE program order per step is staged so the tensor engine always has
ready work while ACT/DVE run the attention chain:
  Wh | U(kt0,1) | prj(b01) | U(kt2) | prj(b23) | U(kt3) | sum | wctx | V

State is kept doubled (hT=2h, cT=2c; U and W_h_att pre-halved on the
host, history halved in the epilogue) so the whole gate tail is five
DVE/ACT ops; the h_att bias-add reads its PSUM tile directly via a
stride-0 broadcast view, and softmax normalization is folded into the
per-ct wctx PSUM evacuations.
"""

import numpy as np
from contextlib import ExitStack

import concourse.bass as bass
import concourse.mybir as mybir
import concourse.tile as tile
from concourse.bass_utils import run_bass_kernel_spmd

F32 = mybir.dt.float32
BF16 = mybir.dt.bfloat16
AF = mybir.ActivationFunctionType
ALU = mybir.AluOpType

B_LOC, T, DIN, D, C, A, L = 4, 256, 512, 512, 512, 512, 128
KT = 4          # 512/128 k-tiles
MT = 16         # 2048/128 m-tiles of the gate dim
NCORES = 8


def split_multi_waits(nc, max_waits=1):
    """This walrus build rejects >1 sync-wait per instruction on some
    opcodes. Hoist extra waits into standalone EventSemaphore preludes."""
    ctr = [0]
    n_fixed = 0

    def fix_block(blk):
        nonlocal n_fixed
        new_insts = []
        for inst in blk.instructions:
            si = inst.sync_info
            waits = list(si.on_wait) if si is not None else []
            if len(waits) > max_waits:
                for w in waits[:-max_waits]:
                    ctr[0] += 1
                    new_insts.append(mybir.InstEventSemaphore(
                        name=f"I-waitsplit-{ctr[0]}",
                        engine=inst.engine, ins=[], outs=[],
                        sync_info=mybir.SyncInfo(on_wait=[w], on_update=[]),
                    ))
                si.on_wait = waits[-max_waits:]
                n_fixed += 1
            new_insts.append(inst)
        blk.instructions[:] = new_insts

    for f in nc.m.functions:
        for blk in f.blocks:
            fix_block(blk)
    return n_fixed


def build_nc(repeat=1):
    nc = bass.Bass()
    x_d = nc.dram_tensor("x", [B_LOC, T, DIN], F32, kind="ExternalInput")
    ctx_d = nc.dram_tensor("context", [B_LOC, L, C], BF16, kind="ExternalInput")
    W_d = nc.dram_tensor("W", [DIN, 4 * D], BF16, kind="ExternalInput")
    V_d = nc.dram_tensor("V", [C, 4 * D], BF16, kind="ExternalInput")
    U_d = nc.dram_tensor("U", [D, 4 * D], BF16, kind="ExternalInput")
    b_d = nc.dram_tensor("b", [4 * D], F32, kind="ExternalInput")
    Wh_d = nc.dram_tensor("W_h_att", [D, A], BF16, kind="ExternalInput")
    Wc_d = nc.dram_tensor("W_ctx_att", [C, A], BF16, kind="ExternalInput")
    ba_d = nc.dram_tensor("b_att", [A], F32, kind="ExternalInput")
    wp_d = nc.dram_tensor("w_att_prj", [A, 1], F32, kind="ExternalInput")
    id_d = nc.dram_tensor("ident", [128, 128], F32, kind="ExternalInput")
    hs_d = nc.dram_tensor("hs", [B_LOC, T, D], F32, kind="ExternalOutput")

    with ExitStack() as ctx:
        tc = ctx.enter_context(tile.TileContext(nc))
        P = ctx.enter_context(tc.tile_pool(name="persist", bufs=1))
        psumP = ctx.enter_context(tc.tile_pool(name="psumP", bufs=1, space="PSUM"))

        # ---------------- persistent tiles ----------------
        xWT = P.tile([128, T * 64], BF16)       # 32KB/part (PE-folded into PSUM)
        hsT = P.tile([128, T * 16], BF16)       # all h_t, col = kt*1024+b*256+t
        idN = P.tile([128, 128], F32)
        idNb = P.tile([128, 128], BF16)
        onesb = P.tile([128, 128], BF16)
        hT = P.tile([128, 16], BF16)            # stores 2*h (see doubled-state note)
        cT = P.tile([128, 16], F32)             # stores 2*c
        expT = P.tile([128, 4], BF16)
        r128 = P.tile([128, 4], F32)
        alphaT = P.tile([128, 4], BF16)
        ctxV = P.tile([128, 8192], BF16)        # (ctx @ V)^T per sample: col = b*2048 + g
        parg = P.tile([128, 2048], BF16)        # actx + h_att (pre-tanh)
        prep = P.tile([128, 2048], BF16)
        garg = P.tile([128, 64], F32)
        dmy = P.tile([128, 4], BF16)            # constant rhs for PE warm-up MMs
        tg = P.tile([128, 64], F32)             # tanh'd gates
        pq = P.tile([128, 32], F32)             # p | q
        tcs = P.tile([128, 16], F32)            # tanh(c)
        bT = P.tile([128, 16], F32)
        batt = P.tile([128, 4], F32)
        w_sb = P.tile([128, 4], BF16)
        actx = P.tile([128, 2048], BF16)

        pp_pre = psumP.tile([128, 64], F32)
        # small attention tiles share one PSUM bank (bank-granular alloc)
        pp_att = psumP.tile([128, 64], F32)
        pp_hatt = pp_att[:, 0:16]
        pp_prjT = pp_att[:, 16:20]
        pp_sum = pp_att[:, 20:24]
        pp_wc = pp_att[:, 24:40]
        pp_trb = psumP.tile([128, 128], BF16)
        pp_dmy = psumP.tile([128, 4], F32)

        nc.vector.memset(hT[:, :], 0.0)
        nc.vector.memset(cT[:, :], 0.0)
        nc.vector.memset(onesb[:, :], 1.0)
        nc.vector.memset(dmy[:, :], 0.125)

        # ---------------- pre-pass (freed afterwards) ----------------
        with tc.tile_pool(name="pre", bufs=1) as PRE, \
             tc.tile_pool(name="psum_pre", bufs=1, space="PSUM") as psumX:
            xT = PRE.tile([128, 4096], BF16)    # col = kt*1024 + b*256 + t
            x_nat = PRE.tile([128, 4096], F32)  # col = (b*2+th)*512 + d
            W_sb = PRE.tile([128, 8192], BF16)  # col = kt*2048 + m
            V_sb = PRE.tile([128, 8192], BF16)  # col = ct*2048 + m
            Wc_sb = PRE.tile([128, 2048], BF16)  # col = ct*512 + a
            ctxT = PRE.tile([128, 2048], BF16)  # col = b*512 + ct*128 + l
            ctx_sb = PRE.tile([128, 2048], BF16)  # col = b*512 + c

            # natural-layout context load (l on partitions, contiguous rows)
            for b_ in range(B_LOC):
                nc.gpsimd.dma_start(ctx_sb[:, b_ * 512:(b_ + 1) * 512],
                                    ctx_d[b_, :, :])
            for ct in range(KT):
                nc.gpsimd.dma_start(V_sb[:, ct * 2048:(ct + 1) * 2048],
                                    V_d[ct * 128:(ct + 1) * 128, :])

            nc.gpsimd.dma_start(idN[:, :], id_d[:, :])
            nc.vector.tensor_copy(idNb[:, :], idN[:, :])
            for b_ in range(B_LOC):
                for th in range(2):
                    nc.gpsimd.dma_start(
                        x_nat[:, (b_ * 2 + th) * 512:(b_ * 2 + th + 1) * 512],
                        x_d[b_, th * 128:(th + 1) * 128, :])
            # on-chip transpose of x: [t, d] blocks -> [d, t] (f32 -> bf16)
            for b_ in range(B_LOC):
                for th in range(2):
                    for kt in range(KT):
                        pt = psumX.tile([128, 128], F32, tag="pa")
                        nc.tensor.transpose(
                            pt[:, :],
                            x_nat[:, (b_ * 2 + th) * 512 + kt * 128:
                                  (b_ * 2 + th) * 512 + (kt + 1) * 128],
                            idN[:, :])
                        nc.vector.tensor_copy(
                            xT[:, kt * 1024 + b_ * 256 + th * 128:
                               kt * 1024 + b_ * 256 + (th + 1) * 128],
                            pt[:, :])
            for kt in range(KT):
                nc.gpsimd.dma_start(W_sb[:, kt * 2048:(kt + 1) * 2048],
                                    W_d[kt * 128:(kt + 1) * 128, :])
                nc.gpsimd.dma_start(Wc_sb[:, kt * 512:(kt + 1) * 512],
                                    Wc_d[kt * 128:(kt + 1) * 128, :])
            # transposed loads of small vectors via PE (row-major DMA + T)
            bt_nat = PRE.tile([16, 128], F32)
            nc.gpsimd.dma_start(bt_nat[:, :], b_d[:].rearrange("(m p) -> m p", p=128))
            pt = psumX.tile([128, 16], F32, tag="pa")
            nc.tensor.transpose(pt[:, :], bt_nat[:, :], idN[0:16, 0:16])
            nc.vector.tensor_copy(bT[:, :], pt[:, :])
            ba_nat = PRE.tile([4, 128], F32)
            nc.gpsimd.dma_start(ba_nat[:, :], ba_d[:].rearrange("(m p) -> m p", p=128))
            pt = psumX.tile([128, 16], F32, tag="pa")
            nc.tensor.transpose(pt[:, 0:4], ba_nat[:, :], idN[0:4, 0:4])
            nc.vector.tensor_copy(batt[:, :], pt[:, 0:4])
            wp_nat = PRE.tile([4, 128], F32)
            nc.gpsimd.dma_start(wp_nat[:, :],
                                wp_d[:, :].rearrange("(m p) one -> m (p one)", p=128))
            pt = psumX.tile([128, 16], F32, tag="pa")
            nc.tensor.transpose(pt[:, 0:4], wp_nat[:, :], idN[0:4, 0:4])
            nc.vector.tensor_copy(w_sb[:, :], pt[:, 0:4])
            # context transposed (c on partitions) via PE from ctx_sb
            for b_ in range(B_LOC):
                for ct in range(KT):
                    pcb = psumX.tile([128, 128], BF16, tag="pa")
                    nc.tensor.transpose(
                        pcb[:, :],
                        ctx_sb[:, b_ * 512 + ct * 128: b_ * 512 + (ct + 1) * 128],
                        idNb[:, :])
                    nc.vector.tensor_copy(
                        ctxT[:, b_ * 512 + ct * 128: b_ * 512 + (ct + 1) * 128],
                        pcb[:, :])

            # xW^T : per (mtile, b) accumulate over ktiles, N=256 (t)
            xWT3 = xWT[:, :].rearrange("p (t m) -> p t m", m=64)
            for mt in range(MT):
                for b_ in range(B_LOC):
                    px = psumX.tile([128, 256], F32, tag="px")
                    for kt in range(KT):
                        nc.tensor.matmul(
                            px[:, :],
                            lhsT=W_sb[:, kt * 2048 + mt * 128: kt * 2048 + (mt + 1) * 128],
                            rhs=xT[:, kt * 1024 + b_ * 256: kt * 1024 + (b_ + 1) * 256],
                            start=(kt == 0), stop=(kt == KT - 1))
                    # fold LSTM bias b while evacuating PSUM
                    nc.scalar.activation(
                        xWT3[:, :, mt * 4 + b_: mt * 4 + b_ + 1].squeeze(),
                        px[:, :], AF.Identity, bias=bT[:, mt:mt + 1])

            # att_ctx^T = Wctx^T @ ctx^T (+ b_att)
            for b_ in range(B_LOC):
                for at in range(KT):
                    pa = psumX.tile([128, 128], F32, tag="pa")
                    for ct in range(KT):
                        nc.tensor.matmul(
                            pa[:, :],
                            lhsT=Wc_sb[:, ct * 512 + at * 128: ct * 512 + (at + 1) * 128],
                            rhs=ctxT[:, b_ * 512 + ct * 128: b_ * 512 + (ct + 1) * 128],
                            start=(ct == 0), stop=(ct == KT - 1))
                    nc.scalar.activation(
                        actx[:, b_ * 512 + at * 128: b_ * 512 + (at + 1) * 128],
                        pa[:, :], AF.Identity, bias=batt[:, at:at + 1])

            # ctxV[b] = (ctx[b] @ V)^T rows l: folds the V projection of the
            # attention context out of the scan entirely
            for b_ in range(B_LOC):
                for gc in range(4):
                    pv = psumX.tile([128, 512], F32, tag="pv")
                    for ct in range(KT):
                        nc.tensor.matmul(
                            pv[:, :],
                            lhsT=ctxT[:, b_ * 512 + ct * 128: b_ * 512 + (ct + 1) * 128],
                            rhs=V_sb[:, ct * 2048 + gc * 512: ct * 2048 + (gc + 1) * 512],
                            start=(ct == 0), stop=(ct == KT - 1))
                    nc.vector.tensor_copy(
                        ctxV[:, b_ * 2048 + gc * 512: b_ * 2048 + (gc + 1) * 512],
                        pv[:, :])

        # weights for the scan (allocated after pre-pass frees its space)
        WTS = ctx.enter_context(tc.tile_pool(name="wts", bufs=1))
        U_sb = WTS.tile([128, 8192], BF16)
        Wh_sb = WTS.tile([128, 2048], BF16)
        for kt in range(KT):
            nc.gpsimd.dma_start(U_sb[:, kt * 2048:(kt + 1) * 2048],
                                U_d[kt * 128:(kt + 1) * 128, :])
            nc.gpsimd.dma_start(Wh_sb[:, kt * 512:(kt + 1) * 512],
                                Wh_d[kt * 128:(kt + 1) * 128, :])

        # ---------------- the scan ----------------
        import os as _os
        UNROLL = int(_os.environ.get("KERNEL_UNROLL", "8"))
        DUMMY = int(_os.environ.get("KERNEL_DUMMY", "24"))
        TSCAN = int(_os.environ.get("KERNEL_TSCAN", str(T)))

        def u_chunk(kts):
            """U^T h matmuls for the given kt values (16 each)."""
            for kt in kts:
                rhs_h = hT[:, kt * 4:(kt + 1) * 4]
                for mt in range(MT):
                    nc.tensor.matmul(
                        pp_pre[:, mt * 4:(mt + 1) * 4],
                        lhsT=U_sb[:, kt * 2048 + mt * 128: kt * 2048 + (mt + 1) * 128],
                        rhs=rhs_h, start=(kt == 0 and mt == 0), stop=False,
                        skip_group_check=True)

        # broadcast views for the h_att bias add: actx[p, (b at l)] + hatt[p, (at b)]
        actx_v = actx[:, :].rearrange("p (b at l) -> p b at l", b=4, at=4, l=128)
        parg_v = parg[:, :].rearrange("p (b at l) -> p b at l", b=4, at=4, l=128)
        hatt_v = pp_hatt.rearrange("p (at b) -> p b at", at=4, b=4)

        def step_body(t, xoff=None, hoff=None):
            # --- PE: h_att^T = Wh^T h (16) ---
            for kt in range(KT):
                rhs_h = hT[:, kt * 4:(kt + 1) * 4]
                for ma in range(4):
                    nc.tensor.matmul(
                        pp_hatt[:, ma * 4:(ma + 1) * 4],
                        lhsT=Wh_sb[:, kt * 512 + ma * 128: kt * 512 + (ma + 1) * 128],
                        rhs=rhs_h, start=(kt == 0 and ma == 0),
                        stop=(kt == KT - 1 and ma == 3), skip_group_check=True)

            # --- DVE: parg = actx + h_att (broadcast add, all-bf16 for 2x DVE)
            # --- ACT: prep = tanh(parg) (2 wide ops) ---
            for half in range(2):
                bs = slice(2 * half, 2 * half + 2)
                nc.vector.tensor_tensor(
                    parg_v[:, bs], actx_v[:, bs],
                    hatt_v[:, bs, :, None].broadcast_to([128, 2, 4, 128]),
                    ALU.add)
                nc.scalar.activation(prep[:, 1024 * half:1024 * (half + 1)],
                                     parg[:, 1024 * half:1024 * (half + 1)],
                                     AF.Tanh)

            # --- PE: U chunk A fills the prep window ---
            u_chunk((0, 1))

            # --- PE: prj^T[l, b] = sum_a prep[a, l] w[a] (16, N=1),
            # split so b0/b1 fire after the first tanh half ---
            def prj(b_):
                for at in range(KT):
                    nc.tensor.matmul(
                        pp_prjT[:, b_:b_ + 1],
                        lhsT=prep[:, b_ * 512 + at * 128: b_ * 512 + (at + 1) * 128],
                        rhs=w_sb[:, at:at + 1],
                        start=(at == 0), stop=(at == KT - 1),
                        skip_group_check=True)
            prj(0); prj(1)
            u_chunk((2,))
            prj(2); prj(3)
            u_chunk((3,))

            # --- ACT: exp (no max-subtraction; inputs bounded) ---
            nc.scalar.activation(expT[:, :], pp_prjT[:, :], AF.Exp)
            # --- PE: row sums on every partition via all-ones stationary ---
            nc.tensor.matmul(pp_sum[:, :], lhsT=onesb[:, :], rhs=expT[:, :],
                             start=True, stop=True, skip_group_check=True)
            # --- DVE: alpha = exp / sum ---
            nc.vector.reciprocal(r128[:, :], pp_sum[:, :])
            nc.vector.tensor_tensor(alphaT[:, :], expT[:, :], r128[:, :], ALU.mult)

            # --- PE: preact^T += (ctx V)^T alpha (64, N=1) ---
            for b_ in range(B_LOC):
                for mt in range(MT):
                    nc.tensor.matmul(
                        pp_pre[:, mt * 4 + b_: mt * 4 + b_ + 1],
                        lhsT=ctxV[:, b_ * 2048 + mt * 128: b_ * 2048 + (mt + 1) * 128],
                        rhs=alphaT[:, b_:b_ + 1],
                        start=False, stop=(b_ == B_LOC - 1 and mt == MT - 1),
                        skip_group_check=True)

            # --- gates. xWT i/f/o region is prescaled (0.5x + 0.5b), so the
            # whole gate stream needs one stt + one add + ONE tanh.
            # PE warm-up filler: keeps the clock up through the gates window
            for d_ in range(DUMMY):
                nc.tensor.matmul(pp_dmy[:, :],
                                 lhsT=U_sb[:, (d_ % 64) * 128:(d_ % 64) * 128 + 128],
                                 rhs=dmy[:, :], start=True, stop=True,
                                 skip_group_check=True)
            xo = t * 64 if xoff is None else xoff
            nc.vector.tensor_add(garg[:, :], pp_pre[:, :], xWT[:, bass.ds(xo, 64)])
            nc.scalar.activation(tg[:, 0:48], garg[:, 0:48], AF.Tanh, scale=0.5)
            nc.scalar.activation(tg[:, 48:64], garg[:, 48:64], AF.Tanh)
            # Doubled state: cT holds 2c, hT holds 2h (U and W_h_att are
            # pre-halved host-side; the epilogue halves the history).
            #   2c' = 0.5*(tf+1)*(2c) + (ti+1)*cand
            #   2h' = (to+1)*tanh(0.5*(2c'))
            nc.vector.scalar_tensor_tensor(pq[:, 0:16], tg[:, 16:32], 1.0,
                                           cT[:, :], ALU.add, ALU.mult)
            nc.vector.scalar_tensor_tensor(pq[:, 16:32], tg[:, 0:16], 1.0,
                                           tg[:, 48:64], ALU.add, ALU.mult)
            nc.vector.scalar_tensor_tensor(cT[:, :], pq[:, 0:16], 0.5,
                                           pq[:, 16:32], ALU.mult, ALU.add)
            nc.scalar.activation(tcs[:, :], cT[:, :], AF.Tanh, scale=0.5)
            nc.vector.scalar_tensor_tensor(hT[:, :], tg[:, 32:48], 1.0,
                                           tcs[:, :], ALU.add, ALU.mult)

            # --- store h_t into the SBUF history buffer (Pool engine) ---
            hsT4 = hsT[:, :].rearrange("p (k b t) -> p k b t", b=4, t=T)
            nc.gpsimd.tensor_copy(hsT4[:, :, :, bass.ds(t if hoff is None else hoff, 1)].squeeze(), hT[:, :])

        with tc.For_i(0, repeat, 1) as _r, \
             tc.For_i(0, TSCAN // UNROLL, 1) as tb:
            base_x = nc.vector.snap(tb * (UNROLL * 64))
            base_h = nc.gpsimd.snap(tb * UNROLL)
            for u in range(UNROLL):
                step_body(tb * UNROLL + u,
                          xoff=base_x + u * 64, hoff=base_h + u)

        # ---------------- epilogue: transpose h history, store ----------------
        STG = ctx.enter_context(tc.tile_pool(name="stage", bufs=2))
        for b_ in range(B_LOC):
            for th in range(2):
                st = STG.tile([128, 512], F32, tag="st")
                for kt in range(KT):
                    nc.tensor.transpose(
                        pp_trb[:, :],
                        hsT[:, kt * 1024 + b_ * 256 + th * 128:
                            kt * 1024 + b_ * 256 + (th + 1) * 128],
                        idNb[:, :])
                    # halve: hsT holds 2h (doubled-state trick)
                    nc.vector.tensor_scalar_mul(st[:, kt * 128:(kt + 1) * 128],
                                                pp_trb[:, :], 0.5)
                nc.sync.dma_start(hs_d[b_, th * 128:(th + 1) * 128, :], st[:, :])

    split_multi_waits(nc)
    return nc


_NC_CACHE = {}


def _get_nc(repeat=1):
    if repeat not in _NC_CACHE:
        _NC_CACHE[repeat] = build_nc(repeat)
    return _NC_CACHE[repeat]


def kernel(x, context, W, V, U, b, W_h_att, W_ctx_att, b_att, w_att_prj,
           bench_repeat=1, **run_kwargs):
    import ml_dtypes
    BF = ml_dtypes.bfloat16
    nc = _get_nc(bench_repeat)
    f32 = lambda a: np.ascontiguousarray(np.asarray(a), dtype=np.float32)
    bf16 = lambda a: np.ascontiguousarray(np.asarray(a, dtype=np.float32).astype(BF))
    x, context = f32(x), np.asarray(context, dtype=np.float32)
    # U and W_h_att pre-halved: the device state tiles hold 2h/2c
    shared = dict(W=bf16(W), V=bf16(V), U=bf16(np.asarray(U) * 0.5), b=f32(b),
                  W_h_att=bf16(np.asarray(W_h_att) * 0.5), W_ctx_att=bf16(W_ctx_att),
                  b_att=f32(b_att), w_att_prj=f32(w_att_prj),
                  ident=np.eye(128, dtype=np.float32))
    in_maps = []
    for c in range(NCORES):
        m = dict(shared)
        m["x"] = np.ascontiguousarray(x[c * B_LOC:(c + 1) * B_LOC])
        m["context"] = bf16(context[c * B_LOC:(c + 1) * B_LOC])
        in_maps.append(m)
    res = run_bass_kernel_spmd(nc, in_maps, core_ids=list(range(NCORES)),
                               **run_kwargs)
    out = np.concatenate([r["hs"] for r in res.results], axis=0)
    kernel.last_result = res
    return out


if __name__ == "__main__":
    rng = np.random.default_rng(0)
    ins = {
        "x": rng.standard_normal((32, T, DIN), dtype=np.float32),
        "context": rng.standard_normal((32, L, C), dtype=np.float32),
        "W": (rng.standard_normal((DIN, 4 * D), dtype=np.float32) * 0.05),
        "V": (rng.standard_normal((C, 4 * D), dtype=np.float32) * 0.05),
        "U": (rng.standard_normal((D, 4 * D), dtype=np.float32) * 0.05),
        "b": np.zeros(4 * D, np.float32),
        "W_h_att": (rng.standard_normal((D, A), dtype=np.float32) * 0.05),
        "W_ctx_att": (rng.standard_normal((C, A), dtype=np.float32) * 0.05),
        "b_att": np.zeros(A, np.float32),
        "w_att_prj": (rng.standard_normal((A, 1), dtype=np.float32) * 0.05),
    }
    out = kernel(**ins)
    print("out", out.shape, out.dtype, float(np.abs(out).max()))
